# revision 22
# baseline (speedup 1.0000x reference)
"""Trainium2 Bass kernel for nn_AttModel_self_syb (dense transformer, 6 blocks).

Sharding: data-parallel over batch. 16 batches -> 8 NeuronCores x 2 batches
(512 tokens per core), full weights on every core, zero collectives.
The 401k x 300 embedding table is "gather-sharded" on the host: each core only
receives the (512, 300) rows its tokens reference (pure input sharding).

Feature-major on-device dataflow ([feature_partition, token_free]); v and
attention weights token-major. Perf structure (v2):
  - single ACT table set (natural_log_exp_and_others has exp/ln/relu/square/
    copy) -> no ACT_TABLE_LOADs in steady state
  - attention mask applied by accumulating NEG*(1-mask) into the score PSUM
    via an identity-weight matmul; exp() then yields exact zeros -> no DVE
    mask multiply
  - per-head softmax denominators (from a ones-column in v) collected into a
    [16, 512] tile; one reciprocal; broadcast to the o feature layout with
    tiny selection-matrix matmuls on the (otherwise idle) PE
  - LayerNorm: mean/rstd rows broadcast via ones-row PE matmuls (no gpsimd),
    rstd = exp(-0.5*ln(var+eps)) with no table swaps, residual kept
    mean-shifted in place (LN is invariant to per-token shifts)
  - FFN-down final k-group runs m-major so PSUM drains/squares/stat matmuls
    overlap the tail; weight DMAs use 2KB-per-partition tiles in groups of 8
    with a deep (24-buf) pool so the sync DMA queue prefetches ~2 groups ahead
Matmul operands are bf16 (fp32 PSUM accumulation); residual and statistics
stay fp32.
"""

import os
import contextlib

import numpy as np
import ml_dtypes

import concourse.bass as bass
from concourse import bacc
import concourse.mybir as mybir
import concourse.tile as tile
from concourse.bass_utils import run_bass_kernel_spmd

F32 = mybir.dt.float32
F32R = mybir.dt.float32r
BF16 = mybir.dt.bfloat16
AF = mybir.ActivationFunctionType
ALU = mybir.AluOpType

# model dims (hardcoded per problem spec)
B, T, D, H, NB = 16, 256, 1024, 16, 6
V, GD, MLP_H, FF_H = 401000, 300, 2048, 4096
DH = D // H                    # 64
NCORES = 8
BPC = B // NCORES              # 2 batches per core
N = BPC * T                    # 512 tokens per core
SCALE = 1.0 / float(np.sqrt(DH))
EPS = 1e-8
NEG = float(-(2**32) + 1)

CDT = BF16
NPCDT = ml_dtypes.bfloat16

P = 128
DT_TILES = D // P              # 8
FF_TILES = FF_H // P           # 32
HT = T // P                    # 2 key chunks per batch
VH = DH + 1                    # per-head v columns incl. ones column
VCOLS = H * VH                 # 1040

N_BLOCKS = int(os.environ.get("BASS_KERNEL_NBLOCKS", NB))


def _prime_act_tables(arch):
    """Collapse the activation-table choice to natural_log_exp_and_others,
    which contains every function this kernel uses (exp, ln, relu, square,
    copy, identity).  get_activation_tables() is functools.cached and the
    table-load pass reads the cached dict, so removing those functions from
    all other sets makes the pass emit a single table load."""
    try:
        from concourse.hw_specs import get_activation_tables
        tabs = get_activation_tables(arch)
        keep = "natural_log_exp_and_others"
        if keep not in tabs:
            return
        kept = set(tabs[keep])
        for name, s in tabs.items():
            if name != keep:
                s -= kept
    except Exception as e:  # pragma: no cover - best effort
        print(f"act table priming failed: {e}")


def build_graph(use_bv: bool, ln_affine: bool, use_bias: bool):
    nc = bacc.Bacc()
    _prime_act_tables(nc.m.arch)
    g = {}
    g["eT"] = nc.declare_dram_parameter("eT", [GD, N], CDT, isOutput=False)
    g["posT"] = nc.declare_dram_parameter("posT", [D, N], F32, isOutput=False)
    g["maskneg"] = nc.declare_dram_parameter("maskneg", [HT, P, N], CDT, isOutput=False)
    g["qm8"] = nc.declare_dram_parameter("qm8", [H // 2, N], F32, isOutput=False)
    g["sselA"] = nc.declare_dram_parameter("sselA", [H // 2, D // 2], F32R, isOutput=False)
    g["sselB"] = nc.declare_dram_parameter("sselB", [H // 2, D // 2], F32R, isOutput=False)
    g["ident"] = nc.declare_dram_parameter("ident", [P, P], CDT, isOutput=False)
    g["ones_col"] = nc.declare_dram_parameter("ones_col", [P, 1], F32R, isOutput=False)
    g["ones_row"] = nc.declare_dram_parameter("ones_row", [1, P], F32R, isOutput=False)

    g["mlp_w1"] = nc.declare_dram_parameter("mlp_w1", [GD, MLP_H], CDT, isOutput=False)
    g["mlp_b1"] = nc.declare_dram_parameter("mlp_b1", [MLP_H], F32, isOutput=False)
    g["mlp_w2"] = nc.declare_dram_parameter("mlp_w2", [MLP_H, D], CDT, isOutput=False)
    g["mlp_b2"] = nc.declare_dram_parameter("mlp_b2", [D], F32, isOutput=False)

    for nm, shp in (("wq", [NB, D, D]), ("wk", [NB, D, D]), ("wv", [NB, D, D]),
                    ("ff_w1", [NB, D, FF_H]), ("ff_w2", [NB, FF_H, D])):
        g[nm] = nc.declare_dram_parameter(nm, shp, CDT, isOutput=False)
    for nm, shp in (("bq", [NB, D]), ("bk", [NB, D]), ("bv", [NB, D]),
                    ("ff_b1", [NB, FF_H]), ("ff_b2", [NB, D]),
                    ("ln1_g", [NB, D]), ("ln1_b", [NB, D]),
                    ("ln2_g", [NB, D]), ("ln2_b", [NB, D])):
        g[nm] = nc.declare_dram_parameter(nm, shp, F32, isOutput=False)

    g["out"] = nc.declare_dram_parameter("out", [D, N], F32, isOutput=True)

    with tile.TileContext(nc) as tc:
        _body(nc, tc, g, use_bv, ln_affine, use_bias)
    nc.finalize()
    return nc


def _body(nc, tc, g, use_bv, ln_affine, use_bias):
    ctx = contextlib.ExitStack()
    with ctx:
        # ---- SBUF pools (per-partition bytes in comments) ----
        wp = ctx.enter_context(tc.tile_pool(name="wp", bufs=23))      # 2KB*24 = 48KB
        h1p = ctx.enter_context(tc.tile_pool(name="h1p", bufs=1))    # 32KB
        xbp = ctx.enter_context(tc.tile_pool(name="xbp", bufs=1))    # 1KB*8 = 8KB
        qkp = ctx.enter_context(tc.tile_pool(name="qkp", bufs=1))    # 1KB*16 = 16KB
        vp = ctx.enter_context(tc.tile_pool(name="vp", bufs=1))      # ~2KB*4 = 8.2KB
        esp = ctx.enter_context(tc.tile_pool(name="esp", bufs=4))    # 1KB*4 = 4KB
        rp = ctx.enter_context(tc.tile_pool(name="rp", bufs=1))      # 2KB*8 = 16KB
        op = ctx.enter_context(tc.tile_pool(name="op", bufs=1))      # 1KB*8 = 8KB
        sqp = ctx.enter_context(tc.tile_pool(name="sqp", bufs=3))    # 2KB*3 = 6KB
        dp = ctx.enter_context(tc.tile_pool(name="dp", bufs=2))      # 2KB*2 = 4KB
        rowp = ctx.enter_context(tc.tile_pool(name="rowp", bufs=1))  # tiny
        cstp = ctx.enter_context(tc.tile_pool(name="cstp", bufs=2))  # tiny
        onep = ctx.enter_context(tc.tile_pool(name="onep", bufs=1))  # consts
        bcp = ctx.enter_context(tc.tile_pool(name="bcp", bufs=2))    # 2KB*2 pos stream

        # ---- PSUM: 2 banks of general rotation + 3 double-bank tiles ----
        # "mm" tiles are single banks (projection/FFN chains, out-heads, LN
        # stats/broadcasts).  "sc" tiles are 2 contiguous banks: attention
        # score PSUMs (one exp over both key chunks); their 512-col halves
        # double as the extra FFN-down / mlp2 accumulators.
        psp = ctx.enter_context(tc.tile_pool(name="psp", bufs=2, space="PSUM"))
        pscp = ctx.enter_context(tc.tile_pool(name="pscp", bufs=3, space="PSUM"))

        def ps_tile(name):
            return psp.tile([P, N], F32, name=name, tag="mm")

        def sc_tile(name):
            return pscp.tile([P, 2 * N], F32, name=name, tag="sc")

        # ---- first compute inputs (DMA-queue priority: these gate the MLP) --
        GK = [(0, 128), (128, 128), (256, GD - 256)]
        e_tiles = []
        for i, (k0, kn) in enumerate(GK):
            et = onep.tile([P, N], CDT, name=f"et_{i}", tag=f"emb_{i}")
            nc.sync.dma_start(out=et[:kn, :], in_=g["eT"][k0:k0 + kn, :])
            e_tiles.append((et, kn))
        mw1t = []
        for ph in range(2):
            for i, (k0, kn) in enumerate(GK):
                w = wp.tile([P, 1024], CDT, name=f"mw1_{ph}_{i}", tag="w")
                nc.sync.dma_start(out=w[:kn, :],
                                  in_=g["mlp_w1"][k0:k0 + kn, ph * 1024:(ph + 1) * 1024])
                mw1t.append((w, kn))

        # ---- constants ----
        ones_col = onep.tile([P, 1], F32R, name="ones_col", tag="ones_col")
        nc.sync.dma_start(out=ones_col, in_=g["ones_col"][:, :])
        ones_row = onep.tile([1, P], F32R, name="ones_row", tag="ones_row")
        nc.sync.dma_start(out=ones_row, in_=g["ones_row"][:, :])
        ident = onep.tile([P, P], CDT, name="ident", tag="ident")
        nc.sync.dma_start(out=ident, in_=g["ident"][:, :])
        sselA = onep.tile([H // 2, D // 2], F32R, name="sselA", tag="sselA")
        nc.sync.dma_start(out=sselA, in_=g["sselA"][:, :])
        sselB = onep.tile([H // 2, D // 2], F32R, name="sselB", tag="sselB")
        nc.sync.dma_start(out=sselB, in_=g["sselB"][:, :])
        qm8 = onep.tile([H // 2, N], F32, name="qm8", tag="qm8")
        nc.sync.dma_start(out=qm8, in_=g["qm8"][:, :])
        mneg = []
        for kc in range(HT):
            mt = onep.tile([P, N], CDT, name=f"mneg_{kc}", tag=f"mneg_{kc}")
            nc.sync.dma_start(out=mt, in_=g["maskneg"][kc])
            mneg.append(mt)
        eps30 = onep.tile([1, 1], F32, name="eps30", tag="eps30")
        nc.vector.memset(eps30, 1e-30)

        def bias_bundle(vec_ap, ncols, name):
            tl = cstp.tile([P, ncols], F32, name=name, tag="bias_bundle", bufs=6)
            nc.sync.dma_start(out=tl, in_=vec_ap.rearrange("(m p) -> p m", p=P))
            return tl

        # =============== embedding MLP ===============
        mb1 = bias_bundle(g["mlp_b1"][:], MLP_H // P, "mb1") if use_bias else None
        h0 = h1p.tile([P, (MLP_H // P) * N], CDT, name="h0", tag="h1")
        for ph in range(2):
            w1t = mw1t[ph * 3:(ph + 1) * 3]
            for mm in range(8):
                m = ph * 8 + mm
                ps = ps_tile("mlp1_ps")
                for i, (_, kn) in enumerate(GK):
                    nc.tensor.matmul(ps, w1t[i][0][:kn, mm * P:(mm + 1) * P],
                                     e_tiles[i][0][:kn, :],
                                     start=(i == 0), stop=(i == len(GK) - 1))
                if use_bias:
                    nc.scalar.activation(h0[:, m * N:(m + 1) * N], ps, AF.Relu,
                                         bias=mb1[:, m:m + 1])
                else:
                    nc.scalar.activation(h0[:, m * N:(m + 1) * N], ps, AF.Relu)

        mb2 = bias_bundle(g["mlp_b2"][:], DT_TILES, "mb2") if use_bias else None
        MK = MLP_H // P  # 16
        def acc8(prefix):
            """8 full-N accumulators: 2 single-bank tiles + halves of 3
            double-bank tiles (uses all 8 PSUM banks)."""
            accs = [ps_tile(f"{prefix}_a0"), ps_tile(f"{prefix}_a1")]
            for i in range(3):
                t = sc_tile(f"{prefix}_sc{i}")
                accs.append(t[:, 0:N])
                accs.append(t[:, N:2 * N])
            return accs

        pss = acc8("mlp2")
        for kg in range(2):
            w2t = []
            for j in range(8):
                k = kg * 8 + j
                w = wp.tile([P, D], CDT, name=f"mw2_{k}", tag="w")
                nc.sync.dma_start(out=w, in_=g["mlp_w2"][k * P:(k + 1) * P, :])
                w2t.append(w)
            for j in range(8):
                k = kg * 8 + j
                for m in range(DT_TILES):
                    nc.tensor.matmul(pss[m], w2t[j][:, m * P:(m + 1) * P],
                                     h0[:, k * N:(k + 1) * N],
                                     start=(k == 0), stop=(k == MK - 1))
        x_bf = []
        for m in range(DT_TILES):
            pos_m = bcp.tile([P, N], F32, name=f"pos_{m}", tag="pos")
            nc.sync.dma_start(out=pos_m, in_=g["posT"][m * P:(m + 1) * P, :])
            r0 = rp.tile([P, N], F32R, name=f"r0_{m}", tag=f"r_{m}")
            if use_bias:
                nc.vector.scalar_tensor_tensor(r0, pss[m], mb2[:, m:m + 1], pos_m,
                                               op0=ALU.add, op1=ALU.add)
            else:
                nc.vector.tensor_add(r0, pss[m], pos_m)
            xb = xbp.tile([P, N], CDT, name=f"x0b_{m}", tag=f"x_{m}")
            nc.vector.tensor_copy(xb, r0)
            x_bf.append(xb)

        # =============== transformer blocks ===============
        for blk in range(N_BLOCKS):
            bq_b = bias_bundle(g["bq"][blk, :], DT_TILES, f"bq_{blk}") if use_bias else None
            bk_b = bias_bundle(g["bk"][blk, :], DT_TILES, f"bk_{blk}") if use_bias else None

            # ---- q/k projections, feature-major ----
            qT = [qkp.tile([P, N], CDT, name=f"q{blk}_{m}", tag=f"q_{m}") for m in range(DT_TILES)]
            kTt = [qkp.tile([P, N], CDT, name=f"k{blk}_{m}", tag=f"k_{m}") for m in range(DT_TILES)]
            for wname, bb, dst in (("wq", bq_b, qT), ("wk", bk_b, kTt)):
                wt = []
                for k in range(DT_TILES):
                    w = wp.tile([P, D], CDT, name=f"{wname}{blk}_{k}", tag="w")
                    nc.sync.dma_start(out=w, in_=g[wname][blk, k * P:(k + 1) * P, :])
                    wt.append(w)
                for m in range(DT_TILES):
                    ps = ps_tile(f"{wname}_ps")
                    for k in range(DT_TILES):
                        nc.tensor.matmul(ps, wt[k][:, m * P:(m + 1) * P], x_bf[k],
                                         start=(k == 0), stop=(k == DT_TILES - 1))
                    if use_bias:
                        nc.scalar.activation(dst[m], ps, AF.Relu, bias=bb[:, m:m + 1])
                    else:
                        nc.scalar.activation(dst[m], ps, AF.Relu)

            # ---- v projection, token-major, per-head layout with ones cols ----
            wvt = []
            for k in range(DT_TILES):
                w = wp.tile([P, D], CDT, name=f"wv{blk}_{k}", tag="w")
                nc.sync.dma_start(out=w, in_=g["wv"][blk, k * P:(k + 1) * P, :])
                wvt.append(w)
            if use_bv:
                bv_row = rowp.tile([1, D], F32, name=f"bvr_{blk}", tag="row_bv", bufs=1)
                nc.sync.dma_start(out=bv_row, in_=g["bv"][blk:blk + 1, :])
                bv_bc = bcp.tile([P, D], F32, name=f"bvb_{blk}", tag="bc_bv", bufs=2)
                nc.gpsimd.partition_broadcast(bv_bc, bv_row)
            vt = [vp.tile([P, VCOLS], CDT, name=f"v{blk}_{tt}", tag=f"v_{tt}")
                  for tt in range(BPC * HT)]
            for tt in range(BPC * HT):
                ones_ap = vt[tt].rearrange("p (h c) -> p h c", h=H)[:, :, DH:VH]
                nc.vector.memset(ones_ap, 1.0)
                for half in range(2):
                    ps = ps_tile("v_ps")
                    c0 = half * (D // 2)
                    for k in range(DT_TILES):
                        nc.tensor.matmul(ps, x_bf[k][:, tt * P:(tt + 1) * P],
                                         wvt[k][:, c0:c0 + D // 2],
                                         start=(k == 0), stop=(k == DT_TILES - 1))
                    dst = vt[tt].rearrange("p (h c) -> p h c", h=H)[
                        :, half * (H // 2):(half + 1) * (H // 2), 0:DH]
                    src = ps[:, :D // 2]
                    if use_bv:
                        tmp = sqp.tile([P, D // 2], F32, name="v_tmp", tag="sq")
                        nc.vector.tensor_add(tmp, src, bv_bc[:, c0:c0 + D // 2])
                        src = tmp
                    nc.scalar.activation(
                        dst, src.rearrange("p (h c) -> p h c", c=DH), AF.Relu)

            # ---- attention (head pairs; one 2-bank score PSUM per head) ----
            o_acc = [op.tile([P, N], CDT, name=f"o{blk}_{m}", tag=f"o_{m}")
                     for m in range(DT_TILES)]
            denh = [dp.tile([H // 2, N], F32, name=f"den{half}_{blk}",
                            tag=f"den{half}") for half in range(2)]
            r_new = [None] * DT_TILES

            def emit_pair_scores(j):
                """Scores for heads (2j, 2j+1), mask pre-accumulated.  The two
                heads' score matmuls use disjoint PE row groups (K-partitions
                0-63 vs 64-127), as do the two mask halves, so adjacent
                matmuls run concurrently in the array."""
                ft = j
                sA = sc_tile("scA")
                sB = sc_tile("scB")
                for kc in range(HT):
                    c0 = kc * N
                    for s, sc in ((0, sA), (1, sB)):
                        nc.tensor.matmul(sc[:, c0:c0 + N], ident, mneg[kc],
                                         start=True, stop=False, skip_group_check=True)
                    for b in range(BPC):
                        for s, sc in ((0, sA), (1, sB)):
                            fo = s * DH
                            nc.tensor.matmul(
                                sc[:, c0 + b * T:c0 + (b + 1) * T],
                                kTt[ft][fo:fo + DH, b * T + kc * P: b * T + (kc + 1) * P],
                                qT[ft][fo:fo + DH, b * T:(b + 1) * T],
                                start=False, stop=(b == BPC - 1), skip_group_check=True)
                exA = esp.tile([P, 2 * N], CDT, name="expA", tag="es")
                nc.scalar.activation(exA, sA, AF.Exp, scale=SCALE)
                exB = esp.tile([P, 2 * N], CDT, name="expB", tag="es")
                nc.scalar.activation(exB, sB, AF.Exp, scale=SCALE)
                return (exA, exB)

            def emit_pair_out(j, exs):
                for s, ex in ((0, exs[0]), (1, exs[1])):
                    h, ft, fo = 2 * j + s, j, s * DH
                    ob = ps_tile("o_head_ps")
                    for b in range(BPC):
                        for kc in range(HT):
                            nc.tensor.matmul(ob[:VH, b * T:(b + 1) * T],
                                             vt[b * HT + kc][:, h * VH:(h + 1) * VH],
                                             ex[:, kc * N + b * T:kc * N + (b + 1) * T],
                                             start=(kc == 0), stop=(kc == HT - 1))
                    nc.vector.tensor_copy(o_acc[ft][fo:fo + DH, :], ob[0:DH, :])
                    drow = rowp.tile([1, N], F32, name="drow", tag="drow", bufs=4)
                    nc.vector.tensor_scalar_add(drow, ob[DH:VH, :], 1e-30)
                    nc.gpsimd.dma_start(out=denh[h // 8][h % 8:h % 8 + 1, :], in_=drow)

            def emit_norm_half(half, ssel_h):
                """reciprocal+qmask for 8 head denominators, broadcast to the
                o feature layout via selection matmuls, normalize + residual."""
                nc.vector.reciprocal_approx_fast(denh[half], denh[half])
                rdr = dp.tile([H // 2, N], F32R, name=f"rdr{half}_{blk}",
                              tag=f"rdr{half}")
                nc.vector.tensor_mul(rdr, denh[half], qm8)
                for i in range(4):
                    ft = half * 4 + i
                    nb = ps_tile("norm_ps")
                    nc.tensor.matmul(nb, ssel_h[:, i * P:(i + 1) * P],
                                     rdr, start=True, stop=True)
                    nc.vector.tensor_mul(o_acc[ft], o_acc[ft], nb)
                    r1 = rp.tile([P, N], F32R, name=f"r1_{blk}_{ft}", tag=f"r_{ft}")
                    nc.vector.tensor_add(r1, o_acc[ft], x_bf[ft])
                    r_new[ft] = r1

            pending = []
            for j in range(H // 2):
                exs = emit_pair_scores(j)
                pending.append((j, exs))
                if len(pending) > 1:
                    pj, pexs = pending.pop(0)
                    emit_pair_out(pj, pexs)
                    if pj == 3:
                        emit_norm_half(0, sselA)
            for pj, pexs in pending:
                emit_pair_out(pj, pexs)
            emit_norm_half(1, sselB)
            x_bf, _ = _layernorm(nc, g, blk, "ln1", r_new, ones_col, ones_row,
                                 xbp, sqp, rowp, cstp, psp, None, ln_affine)

            # ---- FFN up (4 column passes of 1024) ----
            fb1 = bias_bundle(g["ff_b1"][blk, :], FF_TILES, f"fb1_{blk}") if use_bias else None
            h1 = h1p.tile([P, FF_TILES * N], CDT, name=f"h1_{blk}", tag="h1")
            for ph in range(4):
                w1t = []
                for k in range(DT_TILES):
                    w = wp.tile([P, D], CDT, name=f"fw1_{blk}_{ph}_{k}", tag="w")
                    nc.sync.dma_start(
                        out=w, in_=g["ff_w1"][blk, k * P:(k + 1) * P,
                                              ph * 1024:(ph + 1) * 1024])
                    w1t.append(w)
                for mm in range(8):
                    m = ph * 8 + mm
                    ps = ps_tile("ff1_ps")
                    for k in range(DT_TILES):
                        nc.tensor.matmul(ps, w1t[k][:, mm * P:(mm + 1) * P], x_bf[k],
                                         start=(k == 0), stop=(k == DT_TILES - 1))
                    if use_bias:
                        nc.scalar.activation(h1[:, m * N:(m + 1) * N], ps, AF.Relu,
                                             bias=fb1[:, m:m + 1])
                    else:
                        nc.scalar.activation(h1[:, m * N:(m + 1) * N], ps, AF.Relu)

            # ---- FFN down: kg 0-2 j-major, kg 3 m-major for early drains ----
            fb2 = bias_bundle(g["ff_b2"][blk, :], DT_TILES, f"fb2_{blk}") if use_bias else None
            pss = acc8(f"ff2_{blk}")
            w2_last = None
            for kg in range(4):
                w2t = []
                for j in range(8):
                    k = kg * 8 + j
                    w = wp.tile([P, D], CDT, name=f"fw2_{blk}_{k}", tag="w")
                    nc.sync.dma_start(out=w,
                                      in_=g["ff_w2"][blk, k * P:(k + 1) * P, :])
                    w2t.append(w)
                if kg < 3:
                    for j in range(8):
                        k = kg * 8 + j
                        for m in range(DT_TILES):
                            nc.tensor.matmul(pss[m], w2t[j][:, m * P:(m + 1) * P],
                                             h1[:, k * N:(k + 1) * N],
                                             start=(k == 0), stop=False)
                else:
                    w2_last = w2t
            # last k-group m-major: each pss[m] chain closes early so its
            # drain/square/stat-matmuls overlap the remaining chains.  The
            # sums/sumsq PSUM tiles reuse the slots of pss[0]/pss[1], so they
            # are allocated (and their chains started) only after those two
            # have drained -- otherwise the PE FIFO deadlocks.
            r_new = []
            sq_tiles = []
            sums = sumsq = None
            for m in range(DT_TILES):
                for j in range(8):
                    k = 24 + j
                    nc.tensor.matmul(pss[m], w2_last[j][:, m * P:(m + 1) * P],
                                     h1[:, k * N:(k + 1) * N],
                                     start=False, stop=(j == 7))
                r2 = rp.tile([P, N], F32R, name=f"r2_{blk}_{m}", tag=f"r_{m}")
                if use_bias:
                    t = sqp.tile([P, N], F32, name="ff2t", tag="sq")
                    nc.vector.scalar_tensor_tensor(t, pss[m], fb2[:, m:m + 1],
                                                   x_bf[m], op0=ALU.add, op1=ALU.add)
                    nc.vector.tensor_copy(r2, t)
                else:
                    nc.vector.tensor_add(r2, pss[m], x_bf[m])
                sq = sqp.tile([P, N], F32R, name="ln2sq", tag="sq")
                nc.scalar.square(sq, r2)
                r_new.append(r2)
                sq_tiles.append(sq)
                if m == 1:
                    sums = psp.tile([P, N], F32, name=f"ln2s_{blk}", tag="mm")[0:1, :]
                    sumsq = psp.tile([P, N], F32, name=f"ln2q_{blk}", tag="mm")[0:1, :]
                    for mm_ in (0, 1):
                        nc.tensor.matmul(sums, ones_col, r_new[mm_],
                                         start=(mm_ == 0), stop=False)
                        nc.tensor.matmul(sumsq, ones_col, sq_tiles[mm_],
                                         start=(mm_ == 0), stop=False)
                elif m > 1:
                    nc.tensor.matmul(sums, ones_col, r2,
                                     start=False, stop=(m == DT_TILES - 1))
                    nc.tensor.matmul(sumsq, ones_col, sq,
                                     start=False, stop=(m == DT_TILES - 1))
            last = blk == N_BLOCKS - 1
            x_bf, _ = _layernorm(nc, g, blk, "ln2", r_new, ones_col, ones_row,
                                 xbp, sqp, rowp, cstp, psp,
                                 g["out"] if last else None, ln_affine,
                                 stats=(sums, sumsq))


def _layernorm(nc, g, blk, which, r_tiles, ones_col, ones_row,
               xbp, sqp, rowp, cstp, psp, out_dram, affine, stats=None):
    """LN over the partition (feature) axis.  r_tiles are updated IN PLACE to
    r - mean (the residual stream stays mean-shifted; LN is invariant to
    per-token shifts so downstream statistics are unaffected)."""
    nt = len(r_tiles)
    if affine:
        gb = cstp.tile([P, nt], F32, name=f"{which}g_{blk}", tag="bias_bundle", bufs=6)
        nc.sync.dma_start(out=gb, in_=g[f"{which}_g"][blk, :].rearrange("(m p) -> p m", p=P))
        bb = cstp.tile([P, nt], F32, name=f"{which}b_{blk}", tag="bias_bundle", bufs=6)
        nc.sync.dma_start(out=bb, in_=g[f"{which}_b"][blk, :].rearrange("(m p) -> p m", p=P))

    if stats is None:
        sums = psp.tile([P, N], F32, name=f"{which}s_{blk}", tag="mm")[0:1, :]
        sumsq = psp.tile([P, N], F32, name=f"{which}q_{blk}", tag="mm")[0:1, :]
        for m in range(nt):
            nc.tensor.matmul(sums, ones_col, r_tiles[m],
                             start=(m == 0), stop=(m == nt - 1))
        for m in range(nt):
            s = sqp.tile([P, N], F32R, name=f"{which}_sq", tag="sq")
            nc.scalar.square(s, r_tiles[m])
            nc.tensor.matmul(sumsq, ones_col, s,
                             start=(m == 0), stop=(m == nt - 1))
    else:
        sums, sumsq = stats

    mean = rowp.tile([1, N], F32R, name=f"{which}_mean", tag="row_a", bufs=1)
    nc.scalar.mul(mean, sums, 1.0 / D)
    # b_mean = ones_row.T @ mean  (PE broadcast, one bank)
    bm = psp.tile([P, N], F32, name=f"{which}_bm", tag="mm")
    nc.tensor.matmul(bm, ones_row, mean, start=True, stop=True)
    # var = sumsq/D - mean^2
    t = rowp.tile([1, N], F32, name=f"{which}_t", tag="row_b", bufs=1)
    nc.vector.scalar_tensor_tensor(t, mean, -1.0, mean, op0=ALU.mult, op1=ALU.mult)
    var = rowp.tile([1, N], F32, name=f"{which}_var", tag="row_c", bufs=1)
    nc.vector.scalar_tensor_tensor(var, sumsq, 1.0 / D, t, op0=ALU.mult, op1=ALU.add)
    # r -= b_mean (in place; residual stays shifted)
    for m in range(nt):
        nc.vector.tensor_sub(r_tiles[m], r_tiles[m], bm)
    # rstd = exp(-0.5*ln(var+eps)) -- same ACT table set as softmax exp
    eps_c = rowp.tile([1, 1], F32, name=f"{which}_eps", tag="row_eps", bufs=2)
    nc.vector.memset(eps_c, EPS)
    lnv = rowp.tile([1, N], F32, name=f"{which}_lnv", tag="row_d", bufs=1)
    nc.scalar.activation(lnv, var, AF.Ln, bias=eps_c)
    rstd = rowp.tile([1, N], F32R, name=f"{which}_rstd", tag="row_e", bufs=1)
    nc.scalar.activation(rstd, lnv, AF.Exp, scale=-0.5)
    br = psp.tile([P, N], F32, name=f"{which}_br", tag="mm")
    nc.tensor.matmul(br, ones_row, rstd, start=True, stop=True)

    xb_out = []
    for m in range(nt):
        if out_dram is not None:
            xo = sqp.tile([P, N], F32, name=f"{which}_xo", tag="sq")
            nc.vector.tensor_mul(xo, r_tiles[m], br)
            if affine:
                nc.vector.tensor_scalar(out=xo, in0=xo, scalar1=gb[:, m:m + 1],
                                        scalar2=bb[:, m:m + 1], op0=ALU.mult, op1=ALU.add)
            nc.sync.dma_start(out=out_dram[m * P:(m + 1) * P, :], in_=xo)
            xb_out.append(None)
        else:
            xb = xbp.tile([P, N], CDT, name=f"{which}_xb_{m}", tag=f"x_{m}")
            if affine:
                xf = sqp.tile([P, N], F32, name=f"{which}_xf", tag="sq")
                nc.vector.tensor_mul(xf, r_tiles[m], br)
                nc.vector.tensor_scalar(out=xb, in0=xf, scalar1=gb[:, m:m + 1],
                                        scalar2=bb[:, m:m + 1], op0=ALU.mult, op1=ALU.add)
            else:
                nc.vector.tensor_mul(xb, r_tiles[m], br)
            xb_out.append(xb)
    return xb_out, r_tiles


# ---------------------------------------------------------------------------
# host side
# ---------------------------------------------------------------------------

def _prepare_inputs(inputs):
    ipt = np.asarray(inputs["syb_ipt"]).astype(np.int64)
    emb = np.asarray(inputs["emb_table"], dtype=np.float32)
    smask = np.asarray(inputs["syb_mask"]).astype(np.int32)
    graph = np.asarray(inputs["syb_graph"]).astype(np.int32)

    gathered = emb[ipt]                                   # (B, T, GD)
    km = smask > 0
    M = (graph > 0) & km[:, None, :]                      # (B, Tq, Tk)
    MT = np.transpose(M, (0, 2, 1))                       # (B, Tk, Tq)
    qs = smask.astype(np.float32)

    posT = np.asarray(inputs["pos_table"], np.float32).T  # (D, T)
    posT2 = np.ascontiguousarray(np.tile(posT, (1, BPC)))

    # selection matrices: feature partition p of tile ft belongs to head
    # 2ft + p//64; A covers heads 0-7 (ft 0-3), B heads 8-15 (ft 4-7)
    sselA = np.zeros((H // 2, D // 2), np.float32)
    sselB = np.zeros((H // 2, D // 2), np.float32)
    for i in range(4):
        for p in range(P):
            sselA[2 * i + p // DH, i * P + p] = 1.0
            sselB[2 * i + p // DH, i * P + p] = 1.0

    def cvt(x):
        return np.ascontiguousarray(np.asarray(x, np.float32).astype(NPCDT))

    def f32(x):
        return np.ascontiguousarray(np.asarray(x, np.float32))

    common = {
        "posT": posT2,
        "ones_col": np.ones((P, 1), np.float32),
        "ones_row": np.ones((1, P), np.float32),
        "ident": np.eye(P, dtype=NPCDT),
        "sselA": sselA,
        "sselB": sselB,
        "mlp_w1": cvt(inputs["mlp_w1"]), "mlp_b1": f32(inputs["mlp_b1"]),
        "mlp_w2": cvt(inputs["mlp_w2"]), "mlp_b2": f32(inputs["mlp_b2"]),
        "wq": cvt(inputs["wq"]), "wk": cvt(inputs["wk"]), "wv": cvt(inputs["wv"]),
        "bq": f32(inputs["bq"]), "bk": f32(inputs["bk"]), "bv": f32(inputs["bv"]),
        "ff_w1": cvt(inputs["ff_w1"]), "ff_b1": f32(inputs["ff_b1"]),
        "ff_w2": cvt(inputs["ff_w2"]), "ff_b2": f32(inputs["ff_b2"]),
        "ln1_g": f32(inputs["ln1_g"]), "ln1_b": f32(inputs["ln1_b"]),
        "ln2_g": f32(inputs["ln2_g"]), "ln2_b": f32(inputs["ln2_b"]),
    }
    use_bv = bool(np.any(np.asarray(inputs["bv"]) != 0))
    use_bias = bool(
        np.any(np.asarray(inputs["bq"]) != 0) or np.any(np.asarray(inputs["bk"]) != 0)
        or np.any(np.asarray(inputs["mlp_b1"]) != 0) or np.any(np.asarray(inputs["mlp_b2"]) != 0)
        or np.any(np.asarray(inputs["ff_b1"]) != 0) or np.any(np.asarray(inputs["ff_b2"]) != 0))
    ln_affine = bool(
        np.any(np.asarray(inputs["ln1_g"]) != 1) or np.any(np.asarray(inputs["ln1_b"]) != 0)
        or np.any(np.asarray(inputs["ln2_g"]) != 1) or np.any(np.asarray(inputs["ln2_b"]) != 0))

    in_maps = []
    for c in range(NCORES):
        b0 = c * BPC
        eT_c = np.ascontiguousarray(gathered[b0:b0 + BPC].reshape(N, GD).T).astype(NPCDT)
        # maskneg[kc][p, b*T + q] = NEG * (1 - M[b0+b, q, kc*128+p])
        mn = np.zeros((HT, P, N), np.float32)
        for kc in range(HT):
            for b in range(BPC):
                mn[kc, :, b * T:(b + 1) * T] = np.where(
                    MT[b0 + b, kc * P:(kc + 1) * P, :], 0.0, NEG)
        qm = np.broadcast_to(
            np.concatenate([qs[b0 + b] for b in range(BPC)])[None, :], (H // 2, N))
        in_maps.append({
            "eT": eT_c,
            "maskneg": mn.astype(NPCDT),
            "qm8": np.ascontiguousarray(qm, dtype=np.float32),
            **common,
        })
    return in_maps, use_bv, ln_affine, use_bias


def _ensure_ntff_hook():
    """The agent image's antenv package lacks axon_hooks; synthesize it so
    run_bass_kernel_spmd(trace=True) can NTFF-profile through libaxon."""
    try:
        from antenv.axon_hooks import get_axon_ntff_profile_hook  # noqa: F401
        return
    except ImportError:
        pass
    try:
        import sys
        import types
        import antenv
        from trn_agent_boot.trn_boot import _ntff_profile_via_ctypes
        hook = _ntff_profile_via_ctypes("/opt/axon/libaxon_pjrt.so")
        mod = types.ModuleType("antenv.axon_hooks")
        mod._hook = hook
        mod.get_axon_ntff_profile_hook = lambda: mod._hook
        def _set(h):
            mod._hook = h
        mod.set_axon_ntff_profile_hook = _set
        sys.modules["antenv.axon_hooks"] = mod
        antenv.axon_hooks = mod
    except Exception as e:  # profiling is best-effort
        print(f"ntff hook injection failed: {e}")


def run(inputs, trace=False, tmpdir=None):
    in_maps, use_bv, ln_affine, use_bias = _prepare_inputs(inputs)
    nc = build_graph(use_bv, ln_affine, use_bias)
    if trace:
        _ensure_ntff_hook()
    res = run_bass_kernel_spmd(nc, in_maps, core_ids=list(range(NCORES)),
                               trace=trace, tmpdir=tmpdir)
    out = np.empty((B, T, D), np.float32)
    for c in range(NCORES):
        xT = np.asarray(res.results[c]["out"])            # (D, N)
        out[c * BPC:(c + 1) * BPC] = xT.T.reshape(BPC, T, D)
    return out, res


def kernel(**inputs):
    out, _ = run(inputs, trace=False)
    return out


# revision 24
# speedup vs baseline: 1.0303x; 1.0303x over previous
"""Trainium2 Bass kernel for nn_AttModel_self_syb (dense transformer, 6 blocks).

Sharding: data-parallel over batch. 16 batches -> 8 NeuronCores x 2 batches
(512 tokens per core), full weights on every core, zero collectives.
The 401k x 300 embedding table is "gather-sharded" on the host: each core only
receives the (512, 300) rows its tokens reference (pure input sharding).

Feature-major on-device dataflow ([feature_partition, token_free]); v and
attention weights token-major. Perf structure (v2):
  - single ACT table set (natural_log_exp_and_others has exp/ln/relu/square/
    copy) -> no ACT_TABLE_LOADs in steady state
  - attention mask applied by accumulating NEG*(1-mask) into the score PSUM
    via an identity-weight matmul; exp() then yields exact zeros -> no DVE
    mask multiply
  - per-head softmax denominators (from a ones-column in v) collected into a
    [16, 512] tile; one reciprocal; broadcast to the o feature layout with
    tiny selection-matrix matmuls on the (otherwise idle) PE
  - LayerNorm: mean/rstd rows broadcast via ones-row PE matmuls (no gpsimd),
    rstd = exp(-0.5*ln(var+eps)) with no table swaps, residual kept
    mean-shifted in place (LN is invariant to per-token shifts)
  - FFN-down final k-group runs m-major so PSUM drains/squares/stat matmuls
    overlap the tail; weight DMAs use 2KB-per-partition tiles in groups of 8
    with a deep (24-buf) pool so the sync DMA queue prefetches ~2 groups ahead
Matmul operands are bf16 (fp32 PSUM accumulation); residual and statistics
stay fp32.
"""

import os
import contextlib

import numpy as np
import ml_dtypes

import concourse.bass as bass
from concourse import bacc
import concourse.mybir as mybir
import concourse.tile as tile
from concourse.bass_utils import run_bass_kernel_spmd

F32 = mybir.dt.float32
F32R = mybir.dt.float32r
BF16 = mybir.dt.bfloat16
AF = mybir.ActivationFunctionType
ALU = mybir.AluOpType

# model dims (hardcoded per problem spec)
B, T, D, H, NB = 16, 256, 1024, 16, 6
V, GD, MLP_H, FF_H = 401000, 300, 2048, 4096
DH = D // H                    # 64
NCORES = 8
BPC = B // NCORES              # 2 batches per core
N = BPC * T                    # 512 tokens per core
SCALE = 1.0 / float(np.sqrt(DH))
EPS = 1e-8
NEG = float(-(2**32) + 1)

CDT = BF16
NPCDT = ml_dtypes.bfloat16

P = 128
DT_TILES = D // P              # 8
FF_TILES = FF_H // P           # 32
HT = T // P                    # 2 key chunks per batch
VH = DH + 1                    # per-head v columns incl. ones column
VCOLS = H * VH                 # 1040

N_BLOCKS = int(os.environ.get("BASS_KERNEL_NBLOCKS", NB))


def _prime_act_tables(arch):
    """Collapse the activation-table choice to natural_log_exp_and_others,
    which contains every function this kernel uses (exp, ln, relu, square,
    copy, identity).  get_activation_tables() is functools.cached and the
    table-load pass reads the cached dict, so removing those functions from
    all other sets makes the pass emit a single table load."""
    try:
        from concourse.hw_specs import get_activation_tables
        tabs = get_activation_tables(arch)
        keep = "natural_log_exp_and_others"
        if keep not in tabs:
            return
        kept = set(tabs[keep])
        for name, s in tabs.items():
            if name != keep:
                s -= kept
    except Exception as e:  # pragma: no cover - best effort
        print(f"act table priming failed: {e}")


def build_graph(use_bv: bool, ln_affine: bool, use_bias: bool):
    nc = bacc.Bacc()
    _prime_act_tables(nc.m.arch)
    g = {}
    g["eT"] = nc.declare_dram_parameter("eT", [GD, N], CDT, isOutput=False)
    g["posT"] = nc.declare_dram_parameter("posT", [D, N], F32, isOutput=False)
    g["maskneg"] = nc.declare_dram_parameter("maskneg", [HT, P, N], CDT, isOutput=False)
    g["qm8"] = nc.declare_dram_parameter("qm8", [H // 2, N], F32, isOutput=False)
    g["sselA"] = nc.declare_dram_parameter("sselA", [H // 2, D // 2], F32R, isOutput=False)
    g["sselB"] = nc.declare_dram_parameter("sselB", [H // 2, D // 2], F32R, isOutput=False)
    g["ident"] = nc.declare_dram_parameter("ident", [P, P], CDT, isOutput=False)
    g["ones_col"] = nc.declare_dram_parameter("ones_col", [P, 1], F32R, isOutput=False)
    g["ones_row"] = nc.declare_dram_parameter("ones_row", [1, P], F32R, isOutput=False)

    g["mlp_w1"] = nc.declare_dram_parameter("mlp_w1", [GD, MLP_H], CDT, isOutput=False)
    g["mlp_b1"] = nc.declare_dram_parameter("mlp_b1", [MLP_H], F32, isOutput=False)
    g["mlp_w2"] = nc.declare_dram_parameter("mlp_w2", [MLP_H, D], CDT, isOutput=False)
    g["mlp_b2"] = nc.declare_dram_parameter("mlp_b2", [D], F32, isOutput=False)

    for nm, shp in (("wq", [NB, D, D]), ("wk", [NB, D, D]), ("wv", [NB, D, D]),
                    ("ff_w1", [NB, D, FF_H]), ("ff_w2", [NB, FF_H, D])):
        g[nm] = nc.declare_dram_parameter(nm, shp, CDT, isOutput=False)
    for nm, shp in (("bq", [NB, D]), ("bk", [NB, D]), ("bv", [NB, D]),
                    ("ff_b1", [NB, FF_H]), ("ff_b2", [NB, D]),
                    ("ln1_g", [NB, D]), ("ln1_b", [NB, D]),
                    ("ln2_g", [NB, D]), ("ln2_b", [NB, D])):
        g[nm] = nc.declare_dram_parameter(nm, shp, F32, isOutput=False)

    g["out"] = nc.declare_dram_parameter("out", [D, N], F32, isOutput=True)

    with tile.TileContext(nc) as tc:
        _body(nc, tc, g, use_bv, ln_affine, use_bias)
    nc.finalize()
    return nc


def _body(nc, tc, g, use_bv, ln_affine, use_bias):
    ctx = contextlib.ExitStack()
    with ctx:
        # ---- SBUF pools (per-partition bytes in comments) ----
        wp = ctx.enter_context(tc.tile_pool(name="wp", bufs=23))      # 2KB*24 = 48KB
        h1p = ctx.enter_context(tc.tile_pool(name="h1p", bufs=1))    # 32KB
        xbp = ctx.enter_context(tc.tile_pool(name="xbp", bufs=1))    # 1KB*8 = 8KB
        qkp = ctx.enter_context(tc.tile_pool(name="qkp", bufs=1))    # 1KB*16 = 16KB
        vp = ctx.enter_context(tc.tile_pool(name="vp", bufs=1))      # ~2KB*4 = 8.2KB
        esp = ctx.enter_context(tc.tile_pool(name="esp", bufs=4))    # 1KB*4 = 4KB
        rp = ctx.enter_context(tc.tile_pool(name="rp", bufs=1))      # 2KB*8 = 16KB
        op = ctx.enter_context(tc.tile_pool(name="op", bufs=1))      # 1KB*8 = 8KB
        sqp = ctx.enter_context(tc.tile_pool(name="sqp", bufs=3))    # 2KB*3 = 6KB
        dp = ctx.enter_context(tc.tile_pool(name="dp", bufs=2))      # 2KB*2 = 4KB
        rowp = ctx.enter_context(tc.tile_pool(name="rowp", bufs=1))  # tiny
        cstp = ctx.enter_context(tc.tile_pool(name="cstp", bufs=2))  # tiny
        onep = ctx.enter_context(tc.tile_pool(name="onep", bufs=1))  # consts
        bcp = ctx.enter_context(tc.tile_pool(name="bcp", bufs=2))    # 2KB*2 pos stream

        # ---- PSUM: 2 banks of general rotation + 3 double-bank tiles ----
        # "mm" tiles are single banks (projection/FFN chains, out-heads, LN
        # stats/broadcasts).  "sc" tiles are 2 contiguous banks: attention
        # score PSUMs (one exp over both key chunks); their 512-col halves
        # double as the extra FFN-down / mlp2 accumulators.
        psp = ctx.enter_context(tc.tile_pool(name="psp", bufs=2, space="PSUM"))
        pscp = ctx.enter_context(tc.tile_pool(name="pscp", bufs=3, space="PSUM"))

        def ps_tile(name):
            return psp.tile([P, N], F32, name=name, tag="mm")

        def sc_tile(name):
            return pscp.tile([P, 2 * N], F32, name=name, tag="sc")

        # ---- first compute inputs (DMA-queue priority: these gate the MLP) --
        GK = [(0, 128), (128, 128), (256, GD - 256)]
        e_tiles = []
        for i, (k0, kn) in enumerate(GK):
            et = onep.tile([P, N], CDT, name=f"et_{i}", tag=f"emb_{i}")
            nc.sync.dma_start(out=et[:kn, :], in_=g["eT"][k0:k0 + kn, :])
            e_tiles.append((et, kn))
        mw1t = []
        for ph in range(2):
            for i, (k0, kn) in enumerate(GK):
                w = wp.tile([P, 1024], CDT, name=f"mw1_{ph}_{i}", tag="w")
                nc.sync.dma_start(out=w[:kn, :],
                                  in_=g["mlp_w1"][k0:k0 + kn, ph * 1024:(ph + 1) * 1024])
                mw1t.append((w, kn))

        # ---- constants ----
        ones_col = onep.tile([P, 1], F32R, name="ones_col", tag="ones_col")
        nc.sync.dma_start(out=ones_col, in_=g["ones_col"][:, :])
        ones_row = onep.tile([1, P], F32R, name="ones_row", tag="ones_row")
        nc.sync.dma_start(out=ones_row, in_=g["ones_row"][:, :])
        ident = onep.tile([P, P], CDT, name="ident", tag="ident")
        nc.sync.dma_start(out=ident, in_=g["ident"][:, :])
        sselA = onep.tile([H // 2, D // 2], F32R, name="sselA", tag="sselA")
        nc.sync.dma_start(out=sselA, in_=g["sselA"][:, :])
        sselB = onep.tile([H // 2, D // 2], F32R, name="sselB", tag="sselB")
        nc.sync.dma_start(out=sselB, in_=g["sselB"][:, :])
        qm8 = onep.tile([H // 2, N], F32, name="qm8", tag="qm8")
        nc.sync.dma_start(out=qm8, in_=g["qm8"][:, :])
        mneg = []
        for kc in range(HT):
            mt = onep.tile([P, N], CDT, name=f"mneg_{kc}", tag=f"mneg_{kc}")
            nc.sync.dma_start(out=mt, in_=g["maskneg"][kc])
            mneg.append(mt)
        eps30 = onep.tile([1, 1], F32, name="eps30", tag="eps30")
        nc.vector.memset(eps30, 1e-30)

        def bias_bundle(vec_ap, ncols, name):
            tl = cstp.tile([P, ncols], F32, name=name, tag="bias_bundle", bufs=6)
            nc.sync.dma_start(out=tl, in_=vec_ap.rearrange("(m p) -> p m", p=P))
            return tl

        # =============== embedding MLP ===============
        mb1 = bias_bundle(g["mlp_b1"][:], MLP_H // P, "mb1") if use_bias else None
        h0 = h1p.tile([P, (MLP_H // P) * N], CDT, name="h0", tag="h1")
        for ph in range(2):
            w1t = mw1t[ph * 3:(ph + 1) * 3]
            for mm in range(8):
                m = ph * 8 + mm
                ps = ps_tile("mlp1_ps")
                for i, (_, kn) in enumerate(GK):
                    nc.tensor.matmul(ps, w1t[i][0][:kn, mm * P:(mm + 1) * P],
                                     e_tiles[i][0][:kn, :],
                                     start=(i == 0), stop=(i == len(GK) - 1))
                if use_bias:
                    nc.scalar.activation(h0[:, m * N:(m + 1) * N], ps, AF.Relu,
                                         bias=mb1[:, m:m + 1])
                else:
                    nc.scalar.activation(h0[:, m * N:(m + 1) * N], ps, AF.Relu)

        mb2 = bias_bundle(g["mlp_b2"][:], DT_TILES, "mb2") if use_bias else None
        MK = MLP_H // P  # 16
        def acc8(prefix):
            """8 full-N accumulators: 2 single-bank tiles + halves of 3
            double-bank tiles (uses all 8 PSUM banks)."""
            accs = [ps_tile(f"{prefix}_a0"), ps_tile(f"{prefix}_a1")]
            for i in range(3):
                t = sc_tile(f"{prefix}_sc{i}")
                accs.append(t[:, 0:N])
                accs.append(t[:, N:2 * N])
            return accs

        pss = acc8("mlp2")
        for kg in range(2):
            w2t = []
            for j in range(8):
                k = kg * 8 + j
                w = wp.tile([P, D], CDT, name=f"mw2_{k}", tag="w")
                nc.sync.dma_start(out=w, in_=g["mlp_w2"][k * P:(k + 1) * P, :])
                w2t.append(w)
            for j in range(8):
                k = kg * 8 + j
                for m in range(DT_TILES):
                    nc.tensor.matmul(pss[m], w2t[j][:, m * P:(m + 1) * P],
                                     h0[:, k * N:(k + 1) * N],
                                     start=(k == 0), stop=(k == MK - 1))
        x_bf = []
        for m in range(DT_TILES):
            pos_m = bcp.tile([P, N], F32, name=f"pos_{m}", tag="pos")
            nc.sync.dma_start(out=pos_m, in_=g["posT"][m * P:(m + 1) * P, :])
            r0 = rp.tile([P, N], F32R, name=f"r0_{m}", tag=f"r_{m}")
            if use_bias:
                nc.vector.scalar_tensor_tensor(r0, pss[m], mb2[:, m:m + 1], pos_m,
                                               op0=ALU.add, op1=ALU.add)
            else:
                nc.vector.tensor_add(r0, pss[m], pos_m)
            xb = xbp.tile([P, N], CDT, name=f"x0b_{m}", tag=f"x_{m}")
            nc.vector.tensor_copy(xb, r0)
            x_bf.append(xb)

        # =============== transformer blocks ===============
        for blk in range(N_BLOCKS):
            bq_b = bias_bundle(g["bq"][blk, :], DT_TILES, f"bq_{blk}") if use_bias else None
            bk_b = bias_bundle(g["bk"][blk, :], DT_TILES, f"bk_{blk}") if use_bias else None

            # ---- q/k projections, feature-major ----
            qT = [qkp.tile([P, N], CDT, name=f"q{blk}_{m}", tag=f"q_{m}") for m in range(DT_TILES)]
            kTt = [qkp.tile([P, N], CDT, name=f"k{blk}_{m}", tag=f"k_{m}") for m in range(DT_TILES)]
            for wname, bb, dst in (("wq", bq_b, qT), ("wk", bk_b, kTt)):
                wt = []
                for k in range(DT_TILES):
                    w = wp.tile([P, D], CDT, name=f"{wname}{blk}_{k}", tag="w")
                    nc.sync.dma_start(out=w, in_=g[wname][blk, k * P:(k + 1) * P, :])
                    wt.append(w)
                for m in range(DT_TILES):
                    ps = ps_tile(f"{wname}_ps")
                    for k in range(DT_TILES):
                        nc.tensor.matmul(ps, wt[k][:, m * P:(m + 1) * P], x_bf[k],
                                         start=(k == 0), stop=(k == DT_TILES - 1))
                    if use_bias:
                        nc.scalar.activation(dst[m], ps, AF.Relu, bias=bb[:, m:m + 1])
                    else:
                        nc.scalar.activation(dst[m], ps, AF.Relu)

            # ---- v projection, token-major, per-head layout with ones cols ----
            wvt = []
            for k in range(DT_TILES):
                w = wp.tile([P, D], CDT, name=f"wv{blk}_{k}", tag="w")
                nc.sync.dma_start(out=w, in_=g["wv"][blk, k * P:(k + 1) * P, :])
                wvt.append(w)
            if use_bv:
                bv_row = rowp.tile([1, D], F32, name=f"bvr_{blk}", tag="row_bv", bufs=1)
                nc.sync.dma_start(out=bv_row, in_=g["bv"][blk:blk + 1, :])
                bv_bc = bcp.tile([P, D], F32, name=f"bvb_{blk}", tag="bc_bv", bufs=2)
                nc.gpsimd.partition_broadcast(bv_bc, bv_row)
            vt = [vp.tile([P, VCOLS], CDT, name=f"v{blk}_{tt}", tag=f"v_{tt}")
                  for tt in range(BPC * HT)]
            for tt in range(BPC * HT):
                ones_ap = vt[tt].rearrange("p (h c) -> p h c", h=H)[:, :, DH:VH]
                nc.vector.memset(ones_ap, 1.0)
                for half in range(2):
                    ps = ps_tile("v_ps")
                    c0 = half * (D // 2)
                    for k in range(DT_TILES):
                        nc.tensor.matmul(ps, x_bf[k][:, tt * P:(tt + 1) * P],
                                         wvt[k][:, c0:c0 + D // 2],
                                         start=(k == 0), stop=(k == DT_TILES - 1))
                    dst = vt[tt].rearrange("p (h c) -> p h c", h=H)[
                        :, half * (H // 2):(half + 1) * (H // 2), 0:DH]
                    src = ps[:, :D // 2]
                    if use_bv:
                        tmp = sqp.tile([P, D // 2], F32, name="v_tmp", tag="sq")
                        nc.vector.tensor_add(tmp, src, bv_bc[:, c0:c0 + D // 2])
                        src = tmp
                    nc.scalar.activation(
                        dst, src.rearrange("p (h c) -> p h c", c=DH), AF.Relu)

            # ---- attention (head pairs; one 2-bank score PSUM per head) ----
            o_acc = [op.tile([P, N], CDT, name=f"o{blk}_{m}", tag=f"o_{m}")
                     for m in range(DT_TILES)]
            denh = [dp.tile([H // 2, N], F32, name=f"den{half}_{blk}",
                            tag=f"den{half}") for half in range(2)]
            r_new = [None] * DT_TILES

            def emit_pair_scores(j):
                """Scores for heads (2j, 2j+1), mask pre-accumulated.  The two
                heads' score matmuls use disjoint PE row groups (K-partitions
                0-63 vs 64-127), as do the two mask halves, so adjacent
                matmuls run concurrently in the array."""
                ft = j
                sA = sc_tile("scA")
                sB = sc_tile("scB")
                for kc in range(HT):
                    c0 = kc * N
                    for s, sc in ((0, sA), (1, sB)):
                        nc.tensor.matmul(sc[:, c0:c0 + N], ident, mneg[kc],
                                         start=True, stop=False, skip_group_check=True)
                    for b in range(BPC):
                        for s, sc in ((0, sA), (1, sB)):
                            fo = s * DH
                            nc.tensor.matmul(
                                sc[:, c0 + b * T:c0 + (b + 1) * T],
                                kTt[ft][fo:fo + DH, b * T + kc * P: b * T + (kc + 1) * P],
                                qT[ft][fo:fo + DH, b * T:(b + 1) * T],
                                start=False, stop=(b == BPC - 1), skip_group_check=True)
                exA = esp.tile([P, 2 * N], CDT, name="expA", tag="es")
                nc.scalar.activation(exA, sA, AF.Exp, scale=SCALE)
                exB = esp.tile([P, 2 * N], CDT, name="expB", tag="es")
                nc.scalar.activation(exB, sB, AF.Exp, scale=SCALE)
                return (exA, exB)

            def emit_pair_out(j, exs):
                for s, ex in ((0, exs[0]), (1, exs[1])):
                    h, ft, fo = 2 * j + s, j, s * DH
                    ob = ps_tile("o_head_ps")
                    for b in range(BPC):
                        for kc in range(HT):
                            nc.tensor.matmul(ob[:VH, b * T:(b + 1) * T],
                                             vt[b * HT + kc][:, h * VH:(h + 1) * VH],
                                             ex[:, kc * N + b * T:kc * N + (b + 1) * T],
                                             start=(kc == 0), stop=(kc == HT - 1))
                    nc.vector.tensor_copy(o_acc[ft][fo:fo + DH, :], ob[0:DH, :])
                    drow = rowp.tile([1, N], F32, name="drow", tag="drow", bufs=4)
                    nc.vector.tensor_scalar_add(drow, ob[DH:VH, :], 1e-30)
                    nc.gpsimd.dma_start(out=denh[h // 8][h % 8:h % 8 + 1, :], in_=drow)

            def emit_norm_half(half, ssel_h):
                """reciprocal+qmask for 8 head denominators, broadcast to the
                o feature layout via selection matmuls, normalize + residual."""
                nc.vector.reciprocal_approx_fast(denh[half], denh[half])
                rdr = dp.tile([H // 2, N], F32R, name=f"rdr{half}_{blk}",
                              tag=f"rdr{half}")
                nc.vector.tensor_mul(rdr, denh[half], qm8)
                for i in range(4):
                    ft = half * 4 + i
                    nb = ps_tile("norm_ps")
                    nc.tensor.matmul(nb, ssel_h[:, i * P:(i + 1) * P],
                                     rdr, start=True, stop=True)
                    nc.vector.tensor_mul(o_acc[ft], o_acc[ft], nb)
                    r1 = rp.tile([P, N], F32R, name=f"r1_{blk}_{ft}", tag=f"r_{ft}")
                    nc.vector.tensor_add(r1, o_acc[ft], x_bf[ft])
                    r_new[ft] = r1

            pending = []
            for j in range(H // 2):
                exs = emit_pair_scores(j)
                pending.append((j, exs))
                if len(pending) > 1:
                    pj, pexs = pending.pop(0)
                    emit_pair_out(pj, pexs)
                    if pj == 3:
                        emit_norm_half(0, sselA)
            for pj, pexs in pending:
                emit_pair_out(pj, pexs)
            emit_norm_half(1, sselB)
            x_bf, _ = _layernorm(nc, g, blk, "ln1", r_new, ones_col, ones_row,
                                 xbp, sqp, rowp, cstp, psp, None, ln_affine,
                                 variance=(ln_affine or use_bias))

            # ---- FFN up (4 column passes of 1024) ----
            fb1 = bias_bundle(g["ff_b1"][blk, :], FF_TILES, f"fb1_{blk}") if use_bias else None
            h1 = h1p.tile([P, FF_TILES * N], CDT, name=f"h1_{blk}", tag="h1")
            for ph in range(4):
                w1t = []
                for k in range(DT_TILES):
                    w = wp.tile([P, D], CDT, name=f"fw1_{blk}_{ph}_{k}", tag="w")
                    nc.sync.dma_start(
                        out=w, in_=g["ff_w1"][blk, k * P:(k + 1) * P,
                                              ph * 1024:(ph + 1) * 1024])
                    w1t.append(w)
                for mm in range(8):
                    m = ph * 8 + mm
                    ps = ps_tile("ff1_ps")
                    for k in range(DT_TILES):
                        nc.tensor.matmul(ps, w1t[k][:, mm * P:(mm + 1) * P], x_bf[k],
                                         start=(k == 0), stop=(k == DT_TILES - 1))
                    if use_bias:
                        nc.scalar.activation(h1[:, m * N:(m + 1) * N], ps, AF.Relu,
                                             bias=fb1[:, m:m + 1])
                    else:
                        nc.scalar.activation(h1[:, m * N:(m + 1) * N], ps, AF.Relu)

            # ---- FFN down: kg 0-2 j-major, kg 3 m-major for early drains ----
            fb2 = bias_bundle(g["ff_b2"][blk, :], DT_TILES, f"fb2_{blk}") if use_bias else None
            pss = acc8(f"ff2_{blk}")
            w2_last = None
            for kg in range(4):
                w2t = []
                for j in range(8):
                    k = kg * 8 + j
                    w = wp.tile([P, D], CDT, name=f"fw2_{blk}_{k}", tag="w")
                    nc.sync.dma_start(out=w,
                                      in_=g["ff_w2"][blk, k * P:(k + 1) * P, :])
                    w2t.append(w)
                if kg < 3:
                    for j in range(8):
                        k = kg * 8 + j
                        for m in range(DT_TILES):
                            nc.tensor.matmul(pss[m], w2t[j][:, m * P:(m + 1) * P],
                                             h1[:, k * N:(k + 1) * N],
                                             start=(k == 0), stop=False)
                else:
                    w2_last = w2t
            # last k-group m-major: each pss[m] chain closes early so its
            # drain/square/stat-matmuls overlap the remaining chains.  The
            # sums/sumsq PSUM tiles reuse the slots of pss[0]/pss[1], so they
            # are allocated (and their chains started) only after those two
            # have drained -- otherwise the PE FIFO deadlocks.
            r_new = []
            sq_tiles = []
            sums = sumsq = None
            for m in range(DT_TILES):
                for j in range(8):
                    k = 24 + j
                    nc.tensor.matmul(pss[m], w2_last[j][:, m * P:(m + 1) * P],
                                     h1[:, k * N:(k + 1) * N],
                                     start=False, stop=(j == 7))
                r2 = rp.tile([P, N], F32R, name=f"r2_{blk}_{m}", tag=f"r_{m}")
                if use_bias:
                    t = sqp.tile([P, N], F32, name="ff2t", tag="sq")
                    nc.vector.scalar_tensor_tensor(t, pss[m], fb2[:, m:m + 1],
                                                   x_bf[m], op0=ALU.add, op1=ALU.add)
                    nc.vector.tensor_copy(r2, t)
                else:
                    nc.vector.tensor_add(r2, pss[m], x_bf[m])
                sq = sqp.tile([P, N], F32R, name="ln2sq", tag="sq")
                nc.scalar.square(sq, r2)
                r_new.append(r2)
                sq_tiles.append(sq)
                if m == 1:
                    sums = psp.tile([P, N], F32, name=f"ln2s_{blk}", tag="mm")[0:1, :]
                    sumsq = psp.tile([P, N], F32, name=f"ln2q_{blk}", tag="mm")[0:1, :]
                    for mm_ in (0, 1):
                        nc.tensor.matmul(sums, ones_col, r_new[mm_],
                                         start=(mm_ == 0), stop=False)
                        nc.tensor.matmul(sumsq, ones_col, sq_tiles[mm_],
                                         start=(mm_ == 0), stop=False)
                elif m > 1:
                    nc.tensor.matmul(sums, ones_col, r2,
                                     start=False, stop=(m == DT_TILES - 1))
                    nc.tensor.matmul(sumsq, ones_col, sq,
                                     start=False, stop=(m == DT_TILES - 1))
            last = blk == N_BLOCKS - 1
            x_bf, _ = _layernorm(nc, g, blk, "ln2", r_new, ones_col, ones_row,
                                 xbp, sqp, rowp, cstp, psp,
                                 g["out"] if last else None, ln_affine,
                                 stats=(sums, sumsq))


def _layernorm(nc, g, blk, which, r_tiles, ones_col, ones_row,
               xbp, sqp, rowp, cstp, psp, out_dram, affine, stats=None,
               variance=True):
    """LN over the partition (feature) axis.

    variance=False emits a shift-only LN (x~ = r - mean): valid when the LN
    output feeds only bias-free relu-MLP branches and the residual into the
    NEXT LayerNorm -- a positive per-token scale commutes through relu and
    bias-free matmuls and cancels in the next LN, so rstd need never be
    computed.  With variance=True the sub/mul pairs are interleaved per tile
    so the first consumer matmul chain unblocks as early as possible."""
    nt = len(r_tiles)
    if affine:
        gb = cstp.tile([P, nt], F32, name=f"{which}g_{blk}", tag="bias_bundle", bufs=6)
        nc.sync.dma_start(out=gb, in_=g[f"{which}_g"][blk, :].rearrange("(m p) -> p m", p=P))
        bb = cstp.tile([P, nt], F32, name=f"{which}b_{blk}", tag="bias_bundle", bufs=6)
        nc.sync.dma_start(out=bb, in_=g[f"{which}_b"][blk, :].rearrange("(m p) -> p m", p=P))

    if stats is None:
        sums = psp.tile([P, N], F32, name=f"{which}s_{blk}", tag="mm")[0:1, :]
        sumsq = None
        for m in range(nt):
            nc.tensor.matmul(sums, ones_col, r_tiles[m],
                             start=(m == 0), stop=(m == nt - 1))
        if variance:
            sumsq = psp.tile([P, N], F32, name=f"{which}q_{blk}", tag="mm")[0:1, :]
            for m in range(nt):
                s = sqp.tile([P, N], F32R, name=f"{which}_sq", tag="sq")
                nc.scalar.square(s, r_tiles[m])
                nc.tensor.matmul(sumsq, ones_col, s,
                                 start=(m == 0), stop=(m == nt - 1))
    else:
        sums, sumsq = stats

    mean = rowp.tile([1, N], F32R, name=f"{which}_mean", tag="row_a", bufs=1)
    nc.scalar.mul(mean, sums, 1.0 / D)
    # b_mean = ones_row.T @ mean  (PE broadcast, one bank)
    bm = psp.tile([P, N], F32, name=f"{which}_bm", tag="mm")
    nc.tensor.matmul(bm, ones_row, mean, start=True, stop=True)

    if not variance:
        # shift-only: x~ = r - mean, straight to bf16; rstd washes out in the
        # next LN.  No squares / sumsq / rstd chain at all.
        xb_out = []
        for m in range(nt):
            xb = xbp.tile([P, N], CDT, name=f"{which}_xb_{m}", tag=f"x_{m}")
            nc.vector.tensor_sub(xb, r_tiles[m], bm)
            xb_out.append(xb)
        return xb_out, r_tiles

    # var = sumsq/D - mean^2
    t = rowp.tile([1, N], F32, name=f"{which}_t", tag="row_b", bufs=1)
    nc.vector.scalar_tensor_tensor(t, mean, -1.0, mean, op0=ALU.mult, op1=ALU.mult)
    var = rowp.tile([1, N], F32, name=f"{which}_var", tag="row_c", bufs=1)
    nc.vector.scalar_tensor_tensor(var, sumsq, 1.0 / D, t, op0=ALU.mult, op1=ALU.add)
    # rstd = exp(-0.5*ln(var+eps)) -- same ACT table set as softmax exp
    eps_c = rowp.tile([1, 1], F32, name=f"{which}_eps", tag="row_eps", bufs=2)
    nc.vector.memset(eps_c, EPS)
    lnv = rowp.tile([1, N], F32, name=f"{which}_lnv", tag="row_d", bufs=1)
    nc.scalar.activation(lnv, var, AF.Ln, bias=eps_c)
    rstd = rowp.tile([1, N], F32R, name=f"{which}_rstd", tag="row_e", bufs=1)
    nc.scalar.activation(rstd, lnv, AF.Exp, scale=-0.5)
    br = psp.tile([P, N], F32, name=f"{which}_br", tag="mm")
    nc.tensor.matmul(br, ones_row, rstd, start=True, stop=True)

    xb_out = []
    for m in range(nt):
        # interleaved sub/mul per tile: consumer chains unblock per-m
        nc.vector.tensor_sub(r_tiles[m], r_tiles[m], bm)
        if out_dram is not None:
            xo = sqp.tile([P, N], F32, name=f"{which}_xo", tag="sq")
            nc.vector.tensor_mul(xo, r_tiles[m], br)
            if affine:
                nc.vector.tensor_scalar(out=xo, in0=xo, scalar1=gb[:, m:m + 1],
                                        scalar2=bb[:, m:m + 1], op0=ALU.mult, op1=ALU.add)
            nc.sync.dma_start(out=out_dram[m * P:(m + 1) * P, :], in_=xo)
            xb_out.append(None)
        else:
            xb = xbp.tile([P, N], CDT, name=f"{which}_xb_{m}", tag=f"x_{m}")
            if affine:
                xf = sqp.tile([P, N], F32, name=f"{which}_xf", tag="sq")
                nc.vector.tensor_mul(xf, r_tiles[m], br)
                nc.vector.tensor_scalar(out=xb, in0=xf, scalar1=gb[:, m:m + 1],
                                        scalar2=bb[:, m:m + 1], op0=ALU.mult, op1=ALU.add)
            else:
                nc.vector.tensor_mul(xb, r_tiles[m], br)
            xb_out.append(xb)
    return xb_out, r_tiles


# ---------------------------------------------------------------------------
# host side
# ---------------------------------------------------------------------------

def _prepare_inputs(inputs):
    ipt = np.asarray(inputs["syb_ipt"]).astype(np.int64)
    emb = np.asarray(inputs["emb_table"], dtype=np.float32)
    smask = np.asarray(inputs["syb_mask"]).astype(np.int32)
    graph = np.asarray(inputs["syb_graph"]).astype(np.int32)

    gathered = emb[ipt]                                   # (B, T, GD)
    km = smask > 0
    M = (graph > 0) & km[:, None, :]                      # (B, Tq, Tk)
    MT = np.transpose(M, (0, 2, 1))                       # (B, Tk, Tq)
    qs = smask.astype(np.float32)

    posT = np.asarray(inputs["pos_table"], np.float32).T  # (D, T)
    posT2 = np.ascontiguousarray(np.tile(posT, (1, BPC)))

    # selection matrices: feature partition p of tile ft belongs to head
    # 2ft + p//64; A covers heads 0-7 (ft 0-3), B heads 8-15 (ft 4-7)
    sselA = np.zeros((H // 2, D // 2), np.float32)
    sselB = np.zeros((H // 2, D // 2), np.float32)
    for i in range(4):
        for p in range(P):
            sselA[2 * i + p // DH, i * P + p] = 1.0
            sselB[2 * i + p // DH, i * P + p] = 1.0

    def cvt(x):
        return np.ascontiguousarray(np.asarray(x, np.float32).astype(NPCDT))

    def f32(x):
        return np.ascontiguousarray(np.asarray(x, np.float32))

    common = {
        "posT": posT2,
        "ones_col": np.ones((P, 1), np.float32),
        "ones_row": np.ones((1, P), np.float32),
        "ident": np.eye(P, dtype=NPCDT),
        "sselA": sselA,
        "sselB": sselB,
        "mlp_w1": cvt(inputs["mlp_w1"]), "mlp_b1": f32(inputs["mlp_b1"]),
        "mlp_w2": cvt(inputs["mlp_w2"]), "mlp_b2": f32(inputs["mlp_b2"]),
        "wq": cvt(inputs["wq"]), "wk": cvt(inputs["wk"]), "wv": cvt(inputs["wv"]),
        "bq": f32(inputs["bq"]), "bk": f32(inputs["bk"]), "bv": f32(inputs["bv"]),
        "ff_w1": cvt(inputs["ff_w1"]), "ff_b1": f32(inputs["ff_b1"]),
        "ff_w2": cvt(inputs["ff_w2"]), "ff_b2": f32(inputs["ff_b2"]),
        "ln1_g": f32(inputs["ln1_g"]), "ln1_b": f32(inputs["ln1_b"]),
        "ln2_g": f32(inputs["ln2_g"]), "ln2_b": f32(inputs["ln2_b"]),
    }
    use_bv = bool(np.any(np.asarray(inputs["bv"]) != 0))
    use_bias = bool(
        np.any(np.asarray(inputs["bq"]) != 0) or np.any(np.asarray(inputs["bk"]) != 0)
        or np.any(np.asarray(inputs["mlp_b1"]) != 0) or np.any(np.asarray(inputs["mlp_b2"]) != 0)
        or np.any(np.asarray(inputs["ff_b1"]) != 0) or np.any(np.asarray(inputs["ff_b2"]) != 0))
    ln_affine = bool(
        np.any(np.asarray(inputs["ln1_g"]) != 1) or np.any(np.asarray(inputs["ln1_b"]) != 0)
        or np.any(np.asarray(inputs["ln2_g"]) != 1) or np.any(np.asarray(inputs["ln2_b"]) != 0))

    in_maps = []
    for c in range(NCORES):
        b0 = c * BPC
        eT_c = np.ascontiguousarray(gathered[b0:b0 + BPC].reshape(N, GD).T).astype(NPCDT)
        # maskneg[kc][p, b*T + q] = NEG * (1 - M[b0+b, q, kc*128+p])
        mn = np.zeros((HT, P, N), np.float32)
        for kc in range(HT):
            for b in range(BPC):
                mn[kc, :, b * T:(b + 1) * T] = np.where(
                    MT[b0 + b, kc * P:(kc + 1) * P, :], 0.0, NEG)
        qm = np.broadcast_to(
            np.concatenate([qs[b0 + b] for b in range(BPC)])[None, :], (H // 2, N))
        in_maps.append({
            "eT": eT_c,
            "maskneg": mn.astype(NPCDT),
            "qm8": np.ascontiguousarray(qm, dtype=np.float32),
            **common,
        })
    return in_maps, use_bv, ln_affine, use_bias


def _ensure_ntff_hook():
    """The agent image's antenv package lacks axon_hooks; synthesize it so
    run_bass_kernel_spmd(trace=True) can NTFF-profile through libaxon."""
    try:
        from antenv.axon_hooks import get_axon_ntff_profile_hook  # noqa: F401
        return
    except ImportError:
        pass
    try:
        import sys
        import types
        import antenv
        from trn_agent_boot.trn_boot import _ntff_profile_via_ctypes
        hook = _ntff_profile_via_ctypes("/opt/axon/libaxon_pjrt.so")
        mod = types.ModuleType("antenv.axon_hooks")
        mod._hook = hook
        mod.get_axon_ntff_profile_hook = lambda: mod._hook
        def _set(h):
            mod._hook = h
        mod.set_axon_ntff_profile_hook = _set
        sys.modules["antenv.axon_hooks"] = mod
        antenv.axon_hooks = mod
    except Exception as e:  # profiling is best-effort
        print(f"ntff hook injection failed: {e}")


def run(inputs, trace=False, tmpdir=None):
    in_maps, use_bv, ln_affine, use_bias = _prepare_inputs(inputs)
    nc = build_graph(use_bv, ln_affine, use_bias)
    if trace:
        _ensure_ntff_hook()
    res = run_bass_kernel_spmd(nc, in_maps, core_ids=list(range(NCORES)),
                               trace=trace, tmpdir=tmpdir)
    out = np.empty((B, T, D), np.float32)
    for c in range(NCORES):
        xT = np.asarray(res.results[c]["out"])            # (D, N)
        out[c * BPC:(c + 1) * BPC] = xT.T.reshape(BPC, T, D)
    return out, res


def kernel(**inputs):
    out, _ = run(inputs, trace=False)
    return out


# revision 40
# speedup vs baseline: 1.0788x; 1.0471x over previous
"""Trainium2 Bass kernel for nn_AttModel_self_syb (dense transformer, 6 blocks).

Sharding: data-parallel over batch. 16 batches -> 8 NeuronCores x 2 batches
(512 tokens per core), full weights on every core, zero collectives.
The 401k x 300 embedding table is "gather-sharded" on the host: each core only
receives the (512, 300) rows its tokens reference (pure input sharding).

Feature-major on-device dataflow ([feature_partition, token_free]); v and
attention weights token-major. Perf structure (v2):
  - single ACT table set (natural_log_exp_and_others has exp/ln/relu/square/
    copy) -> no ACT_TABLE_LOADs in steady state
  - attention mask applied by accumulating NEG*(1-mask) into the score PSUM
    via an identity-weight matmul; exp() then yields exact zeros -> no DVE
    mask multiply
  - per-head softmax denominators (from a ones-column in v) collected into a
    [16, 512] tile; one reciprocal; broadcast to the o feature layout with
    tiny selection-matrix matmuls on the (otherwise idle) PE
  - LayerNorm: mean/rstd rows broadcast via ones-row PE matmuls (no gpsimd),
    rstd = exp(-0.5*ln(var+eps)) with no table swaps, residual kept
    mean-shifted in place (LN is invariant to per-token shifts)
  - FFN-down final k-group runs m-major so PSUM drains/squares/stat matmuls
    overlap the tail; weight DMAs use 2KB-per-partition tiles in groups of 8
    with a deep (24-buf) pool so the sync DMA queue prefetches ~2 groups ahead
Matmul operands are bf16 (fp32 PSUM accumulation); residual and statistics
stay fp32.
"""

import os
import contextlib

import numpy as np
import ml_dtypes

import concourse.bass as bass
from concourse import bacc
import concourse.mybir as mybir
import concourse.tile as tile
from concourse.bass_utils import run_bass_kernel_spmd

F32 = mybir.dt.float32
F32R = mybir.dt.float32r
BF16 = mybir.dt.bfloat16
AF = mybir.ActivationFunctionType
ALU = mybir.AluOpType

# model dims (hardcoded per problem spec)
B, T, D, H, NB = 16, 256, 1024, 16, 6
V, GD, MLP_H, FF_H = 401000, 300, 2048, 4096
DH = D // H                    # 64
NCORES = 8
BPC = B // NCORES              # 2 batches per core
N = BPC * T                    # 512 tokens per core
SCALE = 1.0 / float(np.sqrt(DH))
EPS = 1e-8
NEG = float(-(2**32) + 1)

CDT = BF16
NPCDT = ml_dtypes.bfloat16

P = 128
DT_TILES = D // P              # 8
FF_TILES = FF_H // P           # 32
HT = T // P                    # 2 key chunks per batch
VH = DH + 1                    # per-head v columns incl. ones column
VCOLS = H * VH                 # 1040

N_BLOCKS = int(os.environ.get("BASS_KERNEL_NBLOCKS", NB))


def _prime_act_tables(arch):
    """Collapse the activation-table choice to natural_log_exp_and_others,
    which contains every function this kernel uses (exp, ln, relu, square,
    copy, identity).  get_activation_tables() is functools.cached and the
    table-load pass reads the cached dict, so removing those functions from
    all other sets makes the pass emit a single table load."""
    try:
        from concourse.hw_specs import get_activation_tables
        tabs = get_activation_tables(arch)
        keep = "natural_log_exp_and_others"
        if keep not in tabs:
            return
        kept = set(tabs[keep])
        for name, s in tabs.items():
            if name != keep:
                s -= kept
    except Exception as e:  # pragma: no cover - best effort
        print(f"act table priming failed: {e}")


def build_graph(use_bv: bool, ln_affine: bool, use_bias: bool):
    nc = bacc.Bacc()
    _prime_act_tables(nc.m.arch)
    g = {}
    g["eT"] = nc.declare_dram_parameter("eT", [GD, N], CDT, isOutput=False)
    g["posT"] = nc.declare_dram_parameter("posT", [D, N], F32, isOutput=False)
    g["maskneg"] = nc.declare_dram_parameter("maskneg", [HT, P, N], CDT, isOutput=False)
    g["qm8"] = nc.declare_dram_parameter("qm8", [H // 2, N], F32, isOutput=False)
    g["sselA"] = nc.declare_dram_parameter("sselA", [H // 2, D // 2], F32R, isOutput=False)
    g["sselB"] = nc.declare_dram_parameter("sselB", [H // 2, D // 2], F32R, isOutput=False)
    g["ident"] = nc.declare_dram_parameter("ident", [P, P], CDT, isOutput=False)
    g["ones_col"] = nc.declare_dram_parameter("ones_col", [P, 1], F32R, isOutput=False)
    g["ones_row"] = nc.declare_dram_parameter("ones_row", [1, P], F32R, isOutput=False)

    g["mlp_w1"] = nc.declare_dram_parameter("mlp_w1", [GD, MLP_H], CDT, isOutput=False)
    g["mlp_b1"] = nc.declare_dram_parameter("mlp_b1", [MLP_H], F32, isOutput=False)
    g["mlp_w2"] = nc.declare_dram_parameter("mlp_w2", [MLP_H, D], CDT, isOutput=False)
    g["mlp_b2"] = nc.declare_dram_parameter("mlp_b2", [D], F32, isOutput=False)

    for nm in ("wq", "wk", "wv"):
        g[nm] = nc.declare_dram_parameter(nm, [NB, D, D], mybir.dt.float8e4,
                                          isOutput=False)
    for nm, shp in (("ff_w1", [NB, D, FF_H]), ("ff_w2", [NB, FF_H, D])):
        g[nm] = nc.declare_dram_parameter(nm, shp, CDT, isOutput=False)
    for nm, shp in (("bq", [NB, D]), ("bk", [NB, D]), ("bv", [NB, D]),
                    ("ff_b1", [NB, FF_H]), ("ff_b2", [NB, D]),
                    ("ln1_g", [NB, D]), ("ln1_b", [NB, D]),
                    ("ln2_g", [NB, D]), ("ln2_b", [NB, D])):
        g[nm] = nc.declare_dram_parameter(nm, shp, F32, isOutput=False)

    g["out"] = nc.declare_dram_parameter("out", [D, N], F32, isOutput=True)

    with tile.TileContext(nc) as tc:
        _body(nc, tc, g, use_bv, ln_affine, use_bias)
    nc.finalize()
    return nc


def _body(nc, tc, g, use_bv, ln_affine, use_bias):
    ctx = contextlib.ExitStack()
    with ctx:
        # ---- SBUF pools (per-partition bytes in comments) ----
        wp = ctx.enter_context(tc.tile_pool(name="wp", bufs=21))      # 2KB*24 = 48KB
        h1p = ctx.enter_context(tc.tile_pool(name="h1p", bufs=1))    # 32KB
        xbp = ctx.enter_context(tc.tile_pool(name="xbp", bufs=1))    # 1KB*8 = 8KB
        qkp = ctx.enter_context(tc.tile_pool(name="qkp", bufs=1))    # 1KB*16 = 16KB
        vp = ctx.enter_context(tc.tile_pool(name="vp", bufs=1))      # ~2KB*4 = 8.2KB
        esp = ctx.enter_context(tc.tile_pool(name="esp", bufs=4))    # 1KB*4 = 4KB
        rp = ctx.enter_context(tc.tile_pool(name="rp", bufs=1))      # 2KB*8 = 16KB
        op = ctx.enter_context(tc.tile_pool(name="op", bufs=1))      # 1KB*8 = 8KB
        sqp = ctx.enter_context(tc.tile_pool(name="sqp", bufs=3))    # 2KB*3 = 6KB
        dp = ctx.enter_context(tc.tile_pool(name="dp", bufs=2))      # 2KB*2 = 4KB
        rowp = ctx.enter_context(tc.tile_pool(name="rowp", bufs=1))  # tiny
        cstp = ctx.enter_context(tc.tile_pool(name="cstp", bufs=2))  # tiny
        onep = ctx.enter_context(tc.tile_pool(name="onep", bufs=1))  # consts
        bcp = ctx.enter_context(tc.tile_pool(name="bcp", bufs=2))    # 2KB*2 pos stream

        # ---- PSUM: 2 banks of general rotation + 3 double-bank tiles ----
        # "mm" tiles are single banks (projection/FFN chains, out-heads, LN
        # stats/broadcasts).  "sc" tiles are 2 contiguous banks: attention
        # score PSUMs (one exp over both key chunks); their 512-col halves
        # double as the extra FFN-down / mlp2 accumulators.
        psp = ctx.enter_context(tc.tile_pool(name="psp", bufs=2, space="PSUM"))
        pscp = ctx.enter_context(tc.tile_pool(name="pscp", bufs=3, space="PSUM"))

        def ps_tile(name):
            return psp.tile([P, N], F32, name=name, tag="mm")

        def sc_tile(name):
            return pscp.tile([P, 2 * N], F32, name=name, tag="sc")

        # ---- first compute inputs (DMA-queue priority: these gate the MLP) --
        GK = [(0, 128), (128, 128), (256, GD - 256)]
        e_tiles = []
        for i, (k0, kn) in enumerate(GK):
            et = onep.tile([P, N], CDT, name=f"et_{i}", tag=f"emb_{i}")
            nc.sync.dma_start(out=et[:kn, :], in_=g["eT"][k0:k0 + kn, :])
            e_tiles.append((et, kn))
        mw1t = []
        for ph in range(2):
            for i, (k0, kn) in enumerate(GK):
                w = wp.tile([P, 1024], CDT, name=f"mw1_{ph}_{i}", tag="w")
                nc.sync.dma_start(out=w[:kn, :],
                                  in_=g["mlp_w1"][k0:k0 + kn, ph * 1024:(ph + 1) * 1024])
                mw1t.append((w, kn))

        # ---- constants ----
        ones_col = onep.tile([P, 1], F32R, name="ones_col", tag="ones_col")
        nc.sync.dma_start(out=ones_col, in_=g["ones_col"][:, :])
        ones_row = onep.tile([1, P], F32R, name="ones_row", tag="ones_row")
        nc.sync.dma_start(out=ones_row, in_=g["ones_row"][:, :])
        ident = onep.tile([P, P], CDT, name="ident", tag="ident")
        nc.sync.dma_start(out=ident, in_=g["ident"][:, :])
        sselA = onep.tile([H // 2, D // 2], F32R, name="sselA", tag="sselA")
        nc.sync.dma_start(out=sselA, in_=g["sselA"][:, :])
        sselB = onep.tile([H // 2, D // 2], F32R, name="sselB", tag="sselB")
        nc.sync.dma_start(out=sselB, in_=g["sselB"][:, :])
        qm8 = onep.tile([H // 2, N], F32, name="qm8", tag="qm8")
        nc.sync.dma_start(out=qm8, in_=g["qm8"][:, :])
        mneg = []
        for kc in range(HT):
            mt = onep.tile([P, N], CDT, name=f"mneg_{kc}", tag=f"mneg_{kc}")
            nc.sync.dma_start(out=mt, in_=g["maskneg"][kc])
            mneg.append(mt)
        eps30 = onep.tile([1, 1], F32, name="eps30", tag="eps30")
        nc.vector.memset(eps30, 1e-30)
        ln8_c = onep.tile([1, 1], F32, name="ln8_c", tag="ln8_c")
        nc.vector.memset(ln8_c, float(np.log(8.0)))
        # fp8 x for the q/k/v DoubleRow matmuls: [part, pair-slot, token]
        x8p = ctx.enter_context(tc.tile_pool(name="x8p", bufs=1))
        F8 = mybir.dt.float8e4
        QKP = DT_TILES // 2

        def bias_bundle(vec_ap, ncols, name):
            tl = cstp.tile([P, ncols], F32, name=name, tag="bias_bundle", bufs=6)
            nc.sync.dma_start(out=tl, in_=vec_ap.rearrange("(m p) -> p m", p=P))
            return tl

        # =============== embedding MLP ===============
        mb1 = bias_bundle(g["mlp_b1"][:], MLP_H // P, "mb1") if use_bias else None
        h0 = h1p.tile([P, (MLP_H // P) * N], CDT, name="h0", tag="h1")
        for ph in range(2):
            w1t = mw1t[ph * 3:(ph + 1) * 3]
            for mm in range(8):
                m = ph * 8 + mm
                ps = ps_tile("mlp1_ps")
                for i, (_, kn) in enumerate(GK):
                    nc.tensor.matmul(ps, w1t[i][0][:kn, mm * P:(mm + 1) * P],
                                     e_tiles[i][0][:kn, :],
                                     start=(i == 0), stop=(i == len(GK) - 1))
                if use_bias:
                    nc.scalar.activation(h0[:, m * N:(m + 1) * N], ps, AF.Relu,
                                         bias=mb1[:, m:m + 1])
                else:
                    nc.scalar.activation(h0[:, m * N:(m + 1) * N], ps, AF.Relu)

        mb2 = bias_bundle(g["mlp_b2"][:], DT_TILES, "mb2") if use_bias else None
        MK = MLP_H // P  # 16
        def acc8(prefix):
            """8 full-N accumulators: 2 single-bank tiles + halves of 3
            double-bank tiles (uses all 8 PSUM banks)."""
            accs = [ps_tile(f"{prefix}_a0"), ps_tile(f"{prefix}_a1")]
            for i in range(3):
                t = sc_tile(f"{prefix}_sc{i}")
                accs.append(t[:, 0:N])
                accs.append(t[:, N:2 * N])
            return accs

        pss = acc8("mlp2")
        for kg in range(2):
            w2t = []
            for j in range(8):
                k = kg * 8 + j
                w = wp.tile([P, D], CDT, name=f"mw2_{k}", tag="w")
                nc.sync.dma_start(out=w, in_=g["mlp_w2"][k * P:(k + 1) * P, :])
                w2t.append(w)
            for j in range(8):
                k = kg * 8 + j
                for m in range(DT_TILES):
                    nc.tensor.matmul(pss[m], w2t[j][:, m * P:(m + 1) * P],
                                     h0[:, k * N:(k + 1) * N],
                                     start=(k == 0), stop=(k == MK - 1))
        x_bf = []
        x8 = [x8p.tile([P, 2, N], F8, name=f"x8_{kp}", tag=f"x8_{kp}")
              for kp in range(QKP)]
        for m in range(DT_TILES):
            pos_m = bcp.tile([P, N], F32, name=f"pos_{m}", tag="pos")
            nc.sync.dma_start(out=pos_m, in_=g["posT"][m * P:(m + 1) * P, :])
            r0 = rp.tile([P, N], F32R, name=f"r0_{m}", tag=f"r_{m}")
            if use_bias:
                nc.vector.scalar_tensor_tensor(r0, pss[m], mb2[:, m:m + 1], pos_m,
                                               op0=ALU.add, op1=ALU.add)
            else:
                nc.vector.tensor_add(r0, pss[m], pos_m)
            nc.vector.tensor_scalar_mul(x8[m // 2][:, m % 2, :], r0, 8.0)
            xb = xbp.tile([P, N], CDT, name=f"x0b_{m}", tag=f"x_{m}")
            nc.vector.tensor_copy(xb, r0)
            x_bf.append(xb)

        # =============== transformer blocks ===============
        for blk in range(N_BLOCKS):
            bq_b = bias_bundle(g["bq"][blk, :], DT_TILES, f"bq_{blk}") if use_bias else None
            bk_b = bias_bundle(g["bk"][blk, :], DT_TILES, f"bk_{blk}") if use_bias else None

            # ---- q/k projections, feature-major, fp8 DoubleRow (x and w are
            # host/LN-scaled by 8 each; the relu activation descales by 1/64)
            DR = mybir.MatmulPerfMode.DoubleRow
            qT = [qkp.tile([P, N], CDT, name=f"q{blk}_{m}", tag=f"q_{m}") for m in range(DT_TILES)]
            kTt = [qkp.tile([P, N], CDT, name=f"k{blk}_{m}", tag=f"k_{m}") for m in range(DT_TILES)]
            for wname, bb, dst in (("wq", bq_b, qT), ("wk", bk_b, kTt)):
                wt = []
                for kp in range(QKP):
                    w = wp.tile([P, 2, D], F8, name=f"{wname}{blk}_{kp}", tag="w")
                    nc.sync.dma_start(
                        out=w, in_=g[wname][blk, kp * 2 * P:(kp + 1) * 2 * P, :]
                        .rearrange("(s p) m -> p s m", p=P))
                    wt.append(w)
                for m in range(DT_TILES):
                    ps = ps_tile(f"{wname}_ps")
                    for kp in range(QKP):
                        nc.tensor.matmul(ps, wt[kp][:, :, m * P:(m + 1) * P], x8[kp],
                                         start=(kp == 0), stop=(kp == QKP - 1),
                                         perf_mode=DR)
                    if use_bias:
                        nc.scalar.activation(dst[m], ps, AF.Relu, scale=1.0 / 64,
                                             bias=bb[:, m:m + 1])
                    else:
                        nc.scalar.activation(dst[m], ps, AF.Relu, scale=1.0 / 64)

            # ---- v projection, token-major, per-head layout with ones cols ----
            wvt = []
            for kp in range(QKP):
                w = wp.tile([P, 2, D], F8, name=f"wv{blk}_{kp}", tag="w")
                nc.sync.dma_start(
                    out=w, in_=g["wv"][blk, kp * 2 * P:(kp + 1) * 2 * P, :]
                    .rearrange("(s p) m -> p s m", p=P))
                wvt.append(w)
            if use_bv:
                bv_row = rowp.tile([1, D], F32, name=f"bvr_{blk}", tag="row_bv", bufs=1)
                nc.sync.dma_start(out=bv_row, in_=g["bv"][blk:blk + 1, :])
                bv_bc = bcp.tile([P, D], F32, name=f"bvb_{blk}", tag="bc_bv", bufs=2)
                nc.gpsimd.partition_broadcast(bv_bc, bv_row)
            vt = [vp.tile([P, VCOLS], CDT, name=f"v{blk}_{tt}", tag=f"v_{tt}")
                  for tt in range(BPC * HT)]
            for tt in range(BPC * HT):
                ones_ap = vt[tt].rearrange("p (h c) -> p h c", h=H)[:, :, DH:VH]
                nc.vector.memset(ones_ap, 1.0)
                for half in range(2):
                    ps = ps_tile("v_ps")
                    c0 = half * (D // 2)
                    for kp in range(QKP):
                        nc.tensor.matmul(ps, x8[kp][:, :, tt * P:(tt + 1) * P],
                                         wvt[kp][:, :, c0:c0 + D // 2],
                                         start=(kp == 0), stop=(kp == QKP - 1),
                                         perf_mode=DR)
                    dst = vt[tt].rearrange("p (h c) -> p h c", h=H)[
                        :, half * (H // 2):(half + 1) * (H // 2), 0:DH]
                    src = ps[:, :D // 2]
                    if use_bv:
                        tmp = sqp.tile([P, D // 2], F32, name="v_tmp", tag="sq")
                        nc.vector.scalar_tensor_tensor(
                            tmp, src, 1.0 / 64, bv_bc[:, c0:c0 + D // 2],
                            op0=ALU.mult, op1=ALU.add)
                        nc.scalar.activation(
                            dst, tmp.rearrange("p (h c) -> p h c", c=DH), AF.Relu)
                    else:
                        nc.scalar.activation(
                            dst, src.rearrange("p (h c) -> p h c", c=DH), AF.Relu,
                            scale=1.0 / 64)

            # ---- attention (head pairs; one 2-bank score PSUM per head) ----
            o_acc = [op.tile([P, N], CDT, name=f"o{blk}_{m}", tag=f"o_{m}")
                     for m in range(DT_TILES)]
            denh = [dp.tile([H // 2, N], F32, name=f"den{half}_{blk}",
                            tag=f"den{half}") for half in range(2)]
            r_new = [None] * DT_TILES

            def emit_pair_scores(j):
                """Scores for heads (2j, 2j+1), mask pre-accumulated.  The two
                heads' score matmuls use disjoint PE row groups (K-partitions
                0-63 vs 64-127), as do the two mask halves, so adjacent
                matmuls run concurrently in the array."""
                ft = j
                sA = sc_tile("scA")
                sB = sc_tile("scB")
                for kc in range(HT):
                    c0 = kc * N
                    for s, sc in ((0, sA), (1, sB)):
                        nc.tensor.matmul(sc[:, c0:c0 + N], ident, mneg[kc],
                                         start=True, stop=False, skip_group_check=True)
                    for b in range(BPC):
                        for s, sc in ((0, sA), (1, sB)):
                            fo = s * DH
                            nc.tensor.matmul(
                                sc[:, c0 + b * T:c0 + (b + 1) * T],
                                kTt[ft][fo:fo + DH, b * T + kc * P: b * T + (kc + 1) * P],
                                qT[ft][fo:fo + DH, b * T:(b + 1) * T],
                                start=False, stop=(b == BPC - 1), skip_group_check=True)
                exA = esp.tile([P, 2 * N], CDT, name="expA", tag="es")
                nc.scalar.activation(exA, sA, AF.Exp, scale=SCALE)
                exB = esp.tile([P, 2 * N], CDT, name="expB", tag="es")
                nc.scalar.activation(exB, sB, AF.Exp, scale=SCALE)
                return (exA, exB)

            def emit_pair_out(j, exs):
                for s, ex in ((0, exs[0]), (1, exs[1])):
                    h, ft, fo = 2 * j + s, j, s * DH
                    ob = ps_tile("o_head_ps")
                    for b in range(BPC):
                        for kc in range(HT):
                            nc.tensor.matmul(ob[:VH, b * T:(b + 1) * T],
                                             vt[b * HT + kc][:, h * VH:(h + 1) * VH],
                                             ex[:, kc * N + b * T:kc * N + (b + 1) * T],
                                             start=(kc == 0), stop=(kc == HT - 1))
                    nc.vector.tensor_copy(o_acc[ft][fo:fo + DH, :], ob[0:DH, :])
                    drow = rowp.tile([1, N], F32, name="drow", tag="drow", bufs=4)
                    nc.vector.tensor_scalar_add(drow, ob[DH:VH, :], 1e-30)
                    nc.gpsimd.dma_start(out=denh[h // 8][h % 8:h % 8 + 1, :], in_=drow)

            def emit_norm_half(half, ssel_h):
                """reciprocal+qmask for 8 head denominators, broadcast to the
                o feature layout via selection matmuls, normalize + residual."""
                nc.vector.reciprocal_approx_fast(denh[half], denh[half])
                rdr = dp.tile([H // 2, N], F32R, name=f"rdr{half}_{blk}",
                              tag=f"rdr{half}")
                nc.vector.tensor_mul(rdr, denh[half], qm8)
                for i in range(4):
                    ft = half * 4 + i
                    nb = ps_tile("norm_ps")
                    nc.tensor.matmul(nb, ssel_h[:, i * P:(i + 1) * P],
                                     rdr, start=True, stop=True)
                    nc.vector.tensor_mul(o_acc[ft], o_acc[ft], nb)
                    r1 = rp.tile([P, N], F32R, name=f"r1_{blk}_{ft}", tag=f"r_{ft}")
                    nc.vector.tensor_add(r1, o_acc[ft], x_bf[ft])
                    r_new[ft] = r1

            pending = []
            for j in range(H // 2):
                exs = emit_pair_scores(j)
                pending.append((j, exs))
                if len(pending) > 1:
                    pj, pexs = pending.pop(0)
                    emit_pair_out(pj, pexs)
                    if pj == 3:
                        emit_norm_half(0, sselA)
            for pj, pexs in pending:
                emit_pair_out(pj, pexs)
            emit_norm_half(1, sselB)
            x_bf, _ = _layernorm(nc, g, blk, "ln1", r_new, ones_col, ones_row,
                                 xbp, sqp, rowp, cstp, psp, None, ln_affine,
                                 variance=(ln_affine or use_bias), bcp=bcp)

            # ---- FFN up (4 column passes of 1024) ----
            fb1 = bias_bundle(g["ff_b1"][blk, :], FF_TILES, f"fb1_{blk}") if use_bias else None
            h1 = h1p.tile([P, FF_TILES * N], CDT, name=f"h1_{blk}", tag="h1")
            for ph in range(4):
                w1t = []
                for k in range(DT_TILES):
                    w = wp.tile([P, D], CDT, name=f"fw1_{blk}_{ph}_{k}", tag="w")
                    nc.sync.dma_start(
                        out=w, in_=g["ff_w1"][blk, k * P:(k + 1) * P,
                                              ph * 1024:(ph + 1) * 1024])
                    w1t.append(w)
                for mm in range(8):
                    m = ph * 8 + mm
                    ps = ps_tile("ff1_ps")
                    for k in range(DT_TILES):
                        nc.tensor.matmul(ps, w1t[k][:, mm * P:(mm + 1) * P], x_bf[k],
                                         start=(k == 0), stop=(k == DT_TILES - 1))
                    if use_bias:
                        nc.scalar.activation(h1[:, m * N:(m + 1) * N], ps, AF.Relu,
                                             bias=fb1[:, m:m + 1])
                    else:
                        nc.scalar.activation(h1[:, m * N:(m + 1) * N], ps, AF.Relu)

            # ---- FFN down: kg 0-2 j-major, kg 3 m-major for early drains ----
            fb2 = bias_bundle(g["ff_b2"][blk, :], DT_TILES, f"fb2_{blk}") if use_bias else None
            pss = acc8(f"ff2_{blk}")
            w2_last = None
            for kg in range(4):
                w2t = []
                for j in range(8):
                    k = kg * 8 + j
                    w = wp.tile([P, D], CDT, name=f"fw2_{blk}_{k}", tag="w")
                    nc.sync.dma_start(out=w,
                                      in_=g["ff_w2"][blk, k * P:(k + 1) * P, :])
                    w2t.append(w)
                if kg < 3:
                    for j in range(8):
                        k = kg * 8 + j
                        for m in range(DT_TILES):
                            nc.tensor.matmul(pss[m], w2t[j][:, m * P:(m + 1) * P],
                                             h1[:, k * N:(k + 1) * N],
                                             start=(k == 0), stop=False)
                else:
                    w2_last = w2t
            # last k-group m-major: each pss[m] chain closes early so its
            # drain/square/stat-matmuls overlap the remaining chains.  The
            # sums/sumsq PSUM tiles reuse the slots of pss[0]/pss[1], so they
            # are allocated (and their chains started) only after those two
            # have drained -- otherwise the PE FIFO deadlocks.
            r_new = []
            sq_tiles = []
            sums = sumsq = None
            for m in range(DT_TILES):
                for j in range(8):
                    k = 24 + j
                    nc.tensor.matmul(pss[m], w2_last[j][:, m * P:(m + 1) * P],
                                     h1[:, k * N:(k + 1) * N],
                                     start=False, stop=(j == 7))
                r2 = rp.tile([P, N], F32R, name=f"r2_{blk}_{m}", tag=f"r_{m}")
                if use_bias:
                    t = sqp.tile([P, N], F32, name="ff2t", tag="sq")
                    nc.vector.scalar_tensor_tensor(t, pss[m], fb2[:, m:m + 1],
                                                   x_bf[m], op0=ALU.add, op1=ALU.add)
                    nc.vector.tensor_copy(r2, t)
                else:
                    nc.vector.tensor_add(r2, pss[m], x_bf[m])
                sq = sqp.tile([P, N], F32R, name="ln2sq", tag="sq")
                nc.scalar.square(sq, r2)
                r_new.append(r2)
                sq_tiles.append(sq)
                if m == 1:
                    sums = psp.tile([P, N], F32, name=f"ln2s_{blk}", tag="mm")[0:1, :]
                    sumsq = psp.tile([P, N], F32, name=f"ln2q_{blk}", tag="mm")[0:1, :]
                    for mm_ in (0, 1):
                        nc.tensor.matmul(sums, ones_col, r_new[mm_],
                                         start=(mm_ == 0), stop=False)
                        nc.tensor.matmul(sumsq, ones_col, sq_tiles[mm_],
                                         start=(mm_ == 0), stop=False)
                elif m > 1:
                    nc.tensor.matmul(sums, ones_col, r2,
                                     start=False, stop=(m == DT_TILES - 1))
                    nc.tensor.matmul(sumsq, ones_col, sq,
                                     start=False, stop=(m == DT_TILES - 1))
            last = blk == N_BLOCKS - 1
            if not last:
                x8 = [x8p.tile([P, 2, N], F8, name=f"x8_{blk}_{kp}", tag=f"x8_{kp}")
                      for kp in range(QKP)]
            x_bf, _ = _layernorm(nc, g, blk, "ln2", r_new, ones_col, ones_row,
                                 xbp, sqp, rowp, cstp, psp,
                                 g["out"] if last else None, ln_affine,
                                 stats=(sums, sumsq), bcp=bcp,
                                 x8_tiles=None if last else x8, ln8_c=ln8_c)


def _layernorm(nc, g, blk, which, r_tiles, ones_col, ones_row,
               xbp, sqp, rowp, cstp, psp, out_dram, affine, stats=None,
               variance=True, bcp=None, x8_tiles=None, ln8_c=None):
    """LN over the partition (feature) axis.

    variance=False emits a shift-only LN (x~ = r - mean): valid when the LN
    output feeds only bias-free relu-MLP branches and the residual into the
    NEXT LayerNorm -- a positive per-token scale commutes through relu and
    bias-free matmuls and cancels in the next LN, so rstd need never be
    computed.  With variance=True the sub/mul pairs are interleaved per tile
    so the first consumer matmul chain unblocks as early as possible."""
    nt = len(r_tiles)
    if affine:
        gb = cstp.tile([P, nt], F32, name=f"{which}g_{blk}", tag="bias_bundle", bufs=6)
        nc.sync.dma_start(out=gb, in_=g[f"{which}_g"][blk, :].rearrange("(m p) -> p m", p=P))
        bb = cstp.tile([P, nt], F32, name=f"{which}b_{blk}", tag="bias_bundle", bufs=6)
        nc.sync.dma_start(out=bb, in_=g[f"{which}_b"][blk, :].rearrange("(m p) -> p m", p=P))

    if stats is None:
        sums = psp.tile([P, N], F32, name=f"{which}s_{blk}", tag="mm")[0:1, :]
        sumsq = None
        for m in range(nt):
            nc.tensor.matmul(sums, ones_col, r_tiles[m],
                             start=(m == 0), stop=(m == nt - 1))
        if variance:
            sumsq = psp.tile([P, N], F32, name=f"{which}q_{blk}", tag="mm")[0:1, :]
            for m in range(nt):
                s = sqp.tile([P, N], F32R, name=f"{which}_sq", tag="sq")
                nc.scalar.square(s, r_tiles[m])
                nc.tensor.matmul(sumsq, ones_col, s,
                                 start=(m == 0), stop=(m == nt - 1))
    else:
        sums, sumsq = stats

    mean = rowp.tile([1, N], F32R, name=f"{which}_mean", tag="row_a", bufs=1)
    nc.scalar.mul(mean, sums, 1.0 / D)
    # broadcasts go through gpsimd into SBUF: no PSUM slot pressure on the
    # apply chain (a PSUM bm/br serializes the next phase's matmul slots
    # against the sub/mul readers).
    bm = bcp.tile([P, N], F32R, name=f"{which}_bm", tag="bmb", bufs=1)
    nc.gpsimd.partition_broadcast(bm, mean)

    if not variance:
        # shift-only: x~ = r - mean, straight to bf16; rstd washes out in the
        # next LN.  No squares / sumsq / rstd chain at all.
        xb_out = []
        for m in range(nt):
            xb = xbp.tile([P, N], CDT, name=f"{which}_xb_{m}", tag=f"x_{m}")
            nc.vector.tensor_sub(xb, r_tiles[m], bm)
            xb_out.append(xb)
        return xb_out, r_tiles

    # var = sumsq/D - mean^2
    t = rowp.tile([1, N], F32, name=f"{which}_t", tag="row_b", bufs=1)
    nc.vector.scalar_tensor_tensor(t, mean, -1.0, mean, op0=ALU.mult, op1=ALU.mult)
    var = rowp.tile([1, N], F32, name=f"{which}_var", tag="row_c", bufs=1)
    nc.vector.scalar_tensor_tensor(var, sumsq, 1.0 / D, t, op0=ALU.mult, op1=ALU.add)
    # rstd = exp(-0.5*ln(var+eps)) -- same ACT table set as softmax exp
    eps_c = rowp.tile([1, 1], F32, name=f"{which}_eps", tag="row_eps", bufs=2)
    nc.vector.memset(eps_c, EPS)
    lnv = rowp.tile([1, N], F32, name=f"{which}_lnv", tag="row_d", bufs=1)
    nc.scalar.activation(lnv, var, AF.Ln, bias=eps_c)
    rstd = rowp.tile([1, N], F32R, name=f"{which}_rstd", tag="row_e", bufs=1)
    nc.scalar.activation(rstd, lnv, AF.Exp, scale=-0.5)
    br = bcp.tile([P, N], F32R, name=f"{which}_br", tag="brb", bufs=1)
    nc.gpsimd.partition_broadcast(br, rstd)
    br8 = None
    if x8_tiles is not None:
        rstd8 = rowp.tile([1, N], F32R, name=f"{which}_rstd8", tag="row_f", bufs=1)
        nc.scalar.activation(rstd8, lnv, AF.Exp, scale=-0.5, bias=ln8_c)
        br8 = bcp.tile([P, N], F32R, name=f"{which}_br8", tag="br8b", bufs=1)
        nc.gpsimd.partition_broadcast(br8, rstd8)

    xb_out = []
    for m in range(nt):
        # interleaved per tile: consumer chains unblock per-m; the fp8 copy
        # (x8 = (r-mean)*rstd*8) comes first since qkv matmuls gate on it
        nc.vector.tensor_sub(r_tiles[m], r_tiles[m], bm)
        if x8_tiles is not None and not affine:
            nc.vector.tensor_mul(x8_tiles[m // 2][:, m % 2, :], r_tiles[m], br8)
        if out_dram is not None:
            xo = sqp.tile([P, N], F32, name=f"{which}_xo", tag="sq")
            nc.vector.tensor_mul(xo, r_tiles[m], br)
            if affine:
                nc.vector.tensor_scalar(out=xo, in0=xo, scalar1=gb[:, m:m + 1],
                                        scalar2=bb[:, m:m + 1], op0=ALU.mult, op1=ALU.add)
            nc.sync.dma_start(out=out_dram[m * P:(m + 1) * P, :], in_=xo)
            xb_out.append(None)
        else:
            xb = xbp.tile([P, N], CDT, name=f"{which}_xb_{m}", tag=f"x_{m}")
            if affine:
                xf = sqp.tile([P, N], F32, name=f"{which}_xf", tag="sq")
                nc.vector.tensor_mul(xf, r_tiles[m], br)
                nc.vector.tensor_scalar(out=xb, in0=xf, scalar1=gb[:, m:m + 1],
                                        scalar2=bb[:, m:m + 1], op0=ALU.mult, op1=ALU.add)
                if x8_tiles is not None:
                    nc.vector.tensor_scalar_mul(x8_tiles[m // 2][:, m % 2, :], xb, 8.0)
            else:
                nc.vector.tensor_mul(xb, r_tiles[m], br)
            xb_out.append(xb)
    return xb_out, r_tiles


# ---------------------------------------------------------------------------
# host side
# ---------------------------------------------------------------------------

def _prepare_inputs(inputs):
    ipt = np.asarray(inputs["syb_ipt"]).astype(np.int64)
    emb = np.asarray(inputs["emb_table"], dtype=np.float32)
    smask = np.asarray(inputs["syb_mask"]).astype(np.int32)
    graph = np.asarray(inputs["syb_graph"]).astype(np.int32)

    gathered = emb[ipt]                                   # (B, T, GD)
    km = smask > 0
    M = (graph > 0) & km[:, None, :]                      # (B, Tq, Tk)
    MT = np.transpose(M, (0, 2, 1))                       # (B, Tk, Tq)
    qs = smask.astype(np.float32)

    posT = np.asarray(inputs["pos_table"], np.float32).T  # (D, T)
    posT2 = np.ascontiguousarray(np.tile(posT, (1, BPC)))

    # selection matrices: feature partition p of tile ft belongs to head
    # 2ft + p//64; A covers heads 0-7 (ft 0-3), B heads 8-15 (ft 4-7)
    sselA = np.zeros((H // 2, D // 2), np.float32)
    sselB = np.zeros((H // 2, D // 2), np.float32)
    for i in range(4):
        for p in range(P):
            sselA[2 * i + p // DH, i * P + p] = 1.0
            sselB[2 * i + p // DH, i * P + p] = 1.0

    def cvt(x):
        return np.ascontiguousarray(np.asarray(x, np.float32).astype(NPCDT))

    def cvt8(x):
        """TRN fp8e4 with x8 scaling (clip to the OCP/TRN-common +-240)."""
        y = np.clip(np.asarray(x, np.float32) * 8.0, -240.0, 240.0)
        return np.ascontiguousarray(y.astype(ml_dtypes.float8_e4m3fn))

    def f32(x):
        return np.ascontiguousarray(np.asarray(x, np.float32))

    common = {
        "posT": posT2,
        "ones_col": np.ones((P, 1), np.float32),
        "ones_row": np.ones((1, P), np.float32),
        "ident": np.eye(P, dtype=NPCDT),
        "sselA": sselA,
        "sselB": sselB,
        "mlp_w1": cvt(inputs["mlp_w1"]), "mlp_b1": f32(inputs["mlp_b1"]),
        "mlp_w2": cvt(inputs["mlp_w2"]), "mlp_b2": f32(inputs["mlp_b2"]),
        "wq": cvt8(inputs["wq"]), "wk": cvt8(inputs["wk"]), "wv": cvt8(inputs["wv"]),
        "bq": f32(inputs["bq"]), "bk": f32(inputs["bk"]), "bv": f32(inputs["bv"]),
        "ff_w1": cvt(inputs["ff_w1"]), "ff_b1": f32(inputs["ff_b1"]),
        "ff_w2": cvt(inputs["ff_w2"]), "ff_b2": f32(inputs["ff_b2"]),
        "ln1_g": f32(inputs["ln1_g"]), "ln1_b": f32(inputs["ln1_b"]),
        "ln2_g": f32(inputs["ln2_g"]), "ln2_b": f32(inputs["ln2_b"]),
    }
    use_bv = bool(np.any(np.asarray(inputs["bv"]) != 0))
    use_bias = bool(
        np.any(np.asarray(inputs["bq"]) != 0) or np.any(np.asarray(inputs["bk"]) != 0)
        or np.any(np.asarray(inputs["mlp_b1"]) != 0) or np.any(np.asarray(inputs["mlp_b2"]) != 0)
        or np.any(np.asarray(inputs["ff_b1"]) != 0) or np.any(np.asarray(inputs["ff_b2"]) != 0))
    ln_affine = bool(
        np.any(np.asarray(inputs["ln1_g"]) != 1) or np.any(np.asarray(inputs["ln1_b"]) != 0)
        or np.any(np.asarray(inputs["ln2_g"]) != 1) or np.any(np.asarray(inputs["ln2_b"]) != 0))

    in_maps = []
    for c in range(NCORES):
        b0 = c * BPC
        eT_c = np.ascontiguousarray(gathered[b0:b0 + BPC].reshape(N, GD).T).astype(NPCDT)
        # maskneg[kc][p, b*T + q] = NEG * (1 - M[b0+b, q, kc*128+p])
        mn = np.zeros((HT, P, N), np.float32)
        for kc in range(HT):
            for b in range(BPC):
                mn[kc, :, b * T:(b + 1) * T] = np.where(
                    MT[b0 + b, kc * P:(kc + 1) * P, :], 0.0, NEG)
        qm = np.broadcast_to(
            np.concatenate([qs[b0 + b] for b in range(BPC)])[None, :], (H // 2, N))
        in_maps.append({
            "eT": eT_c,
            "maskneg": mn.astype(NPCDT),
            "qm8": np.ascontiguousarray(qm, dtype=np.float32),
            **common,
        })
    return in_maps, use_bv, ln_affine, use_bias


def _ensure_ntff_hook():
    """The agent image's antenv package lacks axon_hooks; synthesize it so
    run_bass_kernel_spmd(trace=True) can NTFF-profile through libaxon."""
    try:
        from antenv.axon_hooks import get_axon_ntff_profile_hook  # noqa: F401
        return
    except ImportError:
        pass
    try:
        import sys
        import types
        import antenv
        from trn_agent_boot.trn_boot import _ntff_profile_via_ctypes
        hook = _ntff_profile_via_ctypes("/opt/axon/libaxon_pjrt.so")
        mod = types.ModuleType("antenv.axon_hooks")
        mod._hook = hook
        mod.get_axon_ntff_profile_hook = lambda: mod._hook
        def _set(h):
            mod._hook = h
        mod.set_axon_ntff_profile_hook = _set
        sys.modules["antenv.axon_hooks"] = mod
        antenv.axon_hooks = mod
    except Exception as e:  # profiling is best-effort
        print(f"ntff hook injection failed: {e}")


def run(inputs, trace=False, tmpdir=None):
    in_maps, use_bv, ln_affine, use_bias = _prepare_inputs(inputs)
    nc = build_graph(use_bv, ln_affine, use_bias)
    if trace:
        _ensure_ntff_hook()
    res = run_bass_kernel_spmd(nc, in_maps, core_ids=list(range(NCORES)),
                               trace=trace, tmpdir=tmpdir)
    out = np.empty((B, T, D), np.float32)
    for c in range(NCORES):
        xT = np.asarray(res.results[c]["out"])            # (D, N)
        out[c * BPC:(c + 1) * BPC] = xT.T.reshape(BPC, T, D)
    return out, res


def kernel(**inputs):
    out, _ = run(inputs, trace=False)
    return out
